# revision 1
# baseline (speedup 1.0000x reference)
"""Multi-head causal self-attention (B=2, S=2048, E=1024, H=16, D=64) on 8
Trainium2 NeuronCores.

Sharding: batch x head-group. Core c handles batch (c // 4) and heads
[4*(c%4), 4*(c%4)+4). Each core computes QKV projection for its 4 heads,
causal flash-attention, and a partial output projection over its head
columns. Host sums the 4 partial outputs per batch and adds b_out.

Per-core kernel (matmul inputs bf16, accumulation fp32):
  xT   [E=1024, S=2048]  x[b].T               (contract dim on partitions)
  wqT/wkT/wvT [E, 256]   per-head-group slices of w_qkv, transposed
  woT  [256, E]          w_out[:, head_cols].T
  out  [S, E] fp32       partial output

Attention uses the transpose-free S^T formulation: scores S^T[k, q] =
kT.T @ qT (K = head dim = 64), softmax denominators ride as an appended
ones-column on V (M = 65 ctx matmul), normalization divides by the
denominator row broadcast across partitions on gpsimd. Heads are
processed in pairs with scores/exp/ctx software-pipelined, and the next
chunk's QKV projection + previous chunk's out-projection are interleaved
into the attention waves to keep the PE busy while ACT runs exp.
"""

import sys

if "/opt/trn_rl_repo" not in sys.path:
    sys.path.insert(0, "/opt/trn_rl_repo")

import numpy as np
import ml_dtypes

import concourse.bacc as bacc
import concourse.mybir as mybir
import concourse.tile as tile

BF16 = mybir.dt.bfloat16
FP32 = mybir.dt.float32

B, S, E = 2, 2048, 1024
H, DH = 16, 64
NCORES = 8
HPC = 4            # heads per core
M = HPC * DH       # 256 ctx columns per core
QC = 512           # q chunk
NSC = S // QC      # 4 s-chunks
KB = 128           # k block
SCALE = 1.0 / np.sqrt(DH)


def _emit_kernel(tc, xT, wq, wk, wv, wo_d, out):
    nc = tc.nc
    Exp = mybir.ActivationFunctionType.Exp

    with tc.tile_pool(name="res", bufs=1) as res, \
         tc.tile_pool(name="ps", bufs=1, space="PSUM") as ps, \
         tc.tile_pool(name="expp", bufs=6) as expp, \
         tc.tile_pool(name="scr", bufs=2) as scr, \
         tc.tile_pool(name="outb", bufs=2) as outb:

        # ---- resident SBUF tiles ----
        # all 8 e-chunks of x^T in one tile: [p, e*S + s] (single-DMA loads)
        xt_all = res.tile([128, 8 * S], BF16, name="xt_all")
        xt = [xt_all[:, e * S:(e + 1) * S] for e in range(8)]
        xt_3d = xt_all.rearrange("p (e s) -> p e s", s=S)
        wqt = res.tile([128, 8 * M], BF16, name="wqt")
        wkt = res.tile([128, 8 * M], BF16, name="wkt")
        wvt = res.tile([128, 8 * M], BF16, name="wvt")
        wot = [res.tile([128, E], BF16, name=f"wot{i}") for i in range(2)]
        qTt = [res.tile([128, S], BF16, name=f"qTt{i}") for i in range(2)]
        kTt = [res.tile([128, S], BF16, name=f"kTt{i}") for i in range(2)]
        ctxT = [res.tile([128, S], BF16, name=f"ctxT{i}") for i in range(2)]
        # V with ones column: per (k-block kb, head h) a [128, 65] slab
        v1 = res.tile([128, (S // KB) * HPC * 65], BF16, name="v1")
        v1_3d = v1.rearrange("p (n c) -> p n c", c=65)
        mask = res.tile([128, 128], BF16, name="mask")

        # ---- input DMA: one batched transfer per tensor/chunk (HWDGE
        # generation cost is per dma_start, so batching matters). q/k
        # weights and x chunk 0 first: proj(0) needs them immediately.
        wqt_3d = wqt.rearrange("p (e m) -> p e m", m=M)
        wkt_3d = wkt.rearrange("p (e m) -> p e m", m=M)
        wvt_3d = wvt.rearrange("p (e m) -> p e m", m=M)
        xT_3d = xT.rearrange("(e p) s -> p e s", p=128)
        # startup order: wq -> x[e0-3]c0 -> wk -> x[e4-7]c0 so the first
        # pq matmuls can begin before the whole first chunk lands. (An
        # mt-split of the weight transfers tested neutral: it halves the
        # DMA line width to 256B, doubling per-byte cost.)
        nc.sync.dma_start(wqt_3d[:], wq.rearrange("(e p) m -> p e m", p=128))
        nc.sync.dma_start(xt_3d[:, 0:4, 0:QC], xT_3d[:, 0:4, 0:QC])
        nc.sync.dma_start(wkt_3d[:], wk.rearrange("(e p) m -> p e m", p=128))
        nc.sync.dma_start(xt_3d[:, 4:8, 0:QC], xT_3d[:, 4:8, 0:QC])
        nc.sync.dma_start(wvt_3d[:], wv.rearrange("(e p) m -> p e m", p=128))
        for chunk in range(1, NSC):
            nc.sync.dma_start(
                xt_3d[:, :, chunk * QC:(chunk + 1) * QC],
                xT_3d[:, :, chunk * QC:(chunk + 1) * QC])
        for i in range(2):
            nc.sync.dma_start(wot[i][:], wo_d[i * 128:(i + 1) * 128, :])

        # ---- constants ----
        nc.gpsimd.memset(v1[:], 1.0)  # data columns overwritten by V proj
        # stair mask: keep where k_local <= q_local (within a 128x128 block)
        nc.gpsimd.memset(mask[:], 1.0)
        nc.gpsimd.affine_select(
            out=mask[:], in_=mask[:],
            compare_op=mybir.AluOpType.is_ge,
            fill=0.0, base=0,
            pattern=[[1, 128]],
            channel_multiplier=-1,
        )

        # ---- emission helpers ----
        def emit_proj_qk(sc, mt, wt, dstt, kind):
            s0 = sc * QC
            pqk = ps.tile([128, QC], FP32, tag="proj", name=f"p{kind}_{sc}_{mt}")
            for e in range(8):
                nc.tensor.matmul(
                    pqk[:],
                    lhsT=wt[:, e * M + mt * 128: e * M + (mt + 1) * 128],
                    rhs=xt[e][:, s0:s0 + QC],
                    start=(e == 0), stop=(e == 7))
            nc.vector.tensor_copy(dstt[mt][:, s0:s0 + QC], pqk[:])

        def emit_proj_qk_interleaved(sc, mt):
            # q and k chains interleaved per e-chunk: at kernel start the
            # e-th weight/x DMAs arrive serially, so alternating the two
            # chains doubles the PE work available per arrival.
            s0 = sc * QC
            pq = ps.tile([128, QC], FP32, tag="proj", name=f"pqi_{sc}_{mt}")
            pk = ps.tile([128, QC], FP32, tag="proj", name=f"pki_{sc}_{mt}")
            for e in range(8):
                nc.tensor.matmul(
                    pq[:],
                    lhsT=wqt[:, e * M + mt * 128: e * M + (mt + 1) * 128],
                    rhs=xt[e][:, s0:s0 + QC],
                    start=(e == 0), stop=(e == 7))
                nc.tensor.matmul(
                    pk[:],
                    lhsT=wkt[:, e * M + mt * 128: e * M + (mt + 1) * 128],
                    rhs=xt[e][:, s0:s0 + QC],
                    start=(e == 0), stop=(e == 7))
            nc.vector.tensor_copy(qTt[mt][:, s0:s0 + QC], pq[:])
            nc.vector.tensor_copy(kTt[mt][:, s0:s0 + QC], pk[:])

        def emit_proj_v(sc, sb):
            sblk = sc * 4 + sb
            pv = ps.tile([128, M], FP32, tag="proj", name=f"pv_{sblk}")
            for e in range(8):
                nc.tensor.matmul(
                    pv[:],
                    lhsT=xt[e][:, sblk * 128:(sblk + 1) * 128],
                    rhs=wvt[:, e * M:(e + 1) * M],
                    start=(e == 0), stop=(e == 7))
            nc.vector.tensor_copy(
                v1_3d[:, sblk * HPC:(sblk + 1) * HPC, 0:64],
                pv[:].rearrange("p (h c) -> p h c", c=64))

        def proj_qk_pieces(sc):
            pcs = []
            for mt in range(2):
                pcs.append(lambda mt=mt: emit_proj_qk(sc, mt, wqt, qTt, "q"))
                pcs.append(lambda mt=mt: emit_proj_qk(sc, mt, wkt, kTt, "k"))
            return pcs

        def proj_v_pieces(sc):
            return [lambda sb=sb: emit_proj_v(sc, sb) for sb in range(4)]

        ob_tiles = {}
        out_4d = out.rearrange("(c q p) f -> p c q f", p=128, q=4)

        def emit_outproj(qb, fc):
            sc, qq = qb // 4, qb % 4
            last = sc == NSC - 1
            if (qq == 0 or last) and fc == 0:
                # chunks 0-2: one staging tile + one batched store per chunk
                # (HWDGE generation cost). Chunk 3 stores per q-block so the
                # final store isn't one big exposed transfer at kernel end.
                shape = [128, E] if last else [128, 4 * E]
                ob_tiles[sc] = outb.tile(shape, FP32, tag="ob",
                                         name=f"ob_{qb}")
            ob = ob_tiles[sc]
            po = ps.tile([128, QC], FP32, tag="proj", name=f"po_{qb}_{fc}")
            for mc in range(2):
                nc.tensor.matmul(
                    po[:],
                    lhsT=ctxT[mc][:, qb * 128:(qb + 1) * 128],
                    rhs=wot[mc][:, fc * QC:(fc + 1) * QC],
                    start=(mc == 0), stop=(mc == 1))
            qoff = 0 if last else qq * E
            nc.vector.tensor_copy(
                ob[:, qoff + fc * QC: qoff + (fc + 1) * QC], po[:])
            if fc == 1 and last:
                nc.sync.dma_start(out[qb * 128:(qb + 1) * 128, :], ob[:])
                del ob_tiles[sc]
            elif (qq, fc) == (3, 1):
                nc.sync.dma_start(
                    out_4d[:, sc, :, :],
                    ob.rearrange("p (q f) -> p q f", f=E))
                del ob_tiles[sc]

        def outproj_pieces(sc):
            return [lambda qb=qb, fc=fc: emit_outproj(qb, fc)
                    for qb in range(sc * 4, sc * 4 + 4) for fc in range(2)]

        # ---- attention waves (one head PAIR, one k-block) ----
        # The pair's two heads live on complementary partition halves of the
        # same qT/kT tile (rows 0-63 and 64-127), so their K=64 scores
        # matmuls land on disjoint PE row-groups (tile_position (0,0) and
        # (64,0)) and execute concurrently on hardware.
        def wave_scores(sc, pair, kb):
            s0 = sc * QC
            mt = pair
            rel = kb - 4 * sc
            # Deep-diagonal blocks (rel 2,3) compute/exp only the columns
            # at and above the diagonal, kept at their natural offsets —
            # the two heads' outputs must stay in SEPARATE PSUM banks (the
            # row-tiled score matmuls execute concurrently; same-bank
            # concurrent PE writes fault on hardware).
            lo = rel * 128 if rel >= 2 else 0
            sc_ps = ps.tile([128, 2 * QC], FP32, tag="scores", bufs=2,
                            name=f"s_{sc}_{pair}_{kb}")
            for hh in range(2):
                r0 = hh * 64
                off = hh * QC
                nc.tensor.matmul(
                    sc_ps[:, off + lo: off + QC],
                    lhsT=kTt[mt][r0:r0 + 64, kb * 128:(kb + 1) * 128],
                    rhs=qTt[mt][r0:r0 + 64, s0 + lo: s0 + QC],
                    start=True, stop=True)
            ex = expp.tile([128, 2 * QC], BF16, tag="ex",
                           name=f"e_{sc}_{pair}_{kb}")
            if lo:
                # two calls, one per head: a single strided 3D-AP call
                # tested slower in the cost model (per-row AP overhead
                # exceeds the saved 352-cycle call overhead)
                for hh in range(2):
                    off = hh * QC
                    nc.scalar.activation(ex[:, off + lo: off + QC],
                                         sc_ps[:, off + lo: off + QC],
                                         Exp, scale=SCALE)
            else:
                nc.scalar.activation(ex[:], sc_ps[:], Exp, scale=SCALE)
            if rel >= 0:
                for hh in range(2):
                    off = hh * QC
                    nc.vector.tensor_mul(
                        ex[:, off + rel * 128: off + (rel + 1) * 128],
                        ex[:, off + rel * 128: off + (rel + 1) * 128],
                        mask[:])
            return ex

        def wave_ctx(sc, pair, kb, ex, ctx_pair, nkb):
            rel = kb - 4 * sc
            lo = rel * 128 if rel > 0 else 0
            for hh in range(2):
                h = 2 * pair + hh
                off = hh * QC
                nc.tensor.matmul(
                    ctx_pair[hh][:, lo:QC],
                    lhsT=v1_3d[:, kb * HPC + h, :],
                    rhs=ex[:, off + lo: off + QC],
                    start=(kb == 0), stop=(kb == nkb - 1),
                    skip_group_check=True)

        def emit_norm(sc, h, ctx_ps, stage=True):
            s0 = sc * QC
            mt, r0 = h // 2, (h % 2) * 64
            # recip reads the PSUM denom row directly so it doesn't chain
            # behind the staging copy
            rec = scr.tile([1, QC], FP32, tag="rec", name=f"r_{sc}_{h}")
            nc.vector.reciprocal(rec[:], ctx_ps[64:65, :])
            if stage:
                # stage the accumulated ctx out of PSUM immediately: the
                # PSUM slot is recycled by the next head pair, and holding
                # it through the recip -> gpsimd-broadcast chain stalls the
                # next chunk's first ctx matmuls at every chunk boundary.
                cst = scr.tile([65, QC], FP32, tag="cst", name=f"cs_{sc}_{h}")
                nc.vector.tensor_copy(cst[:], ctx_ps[:])
                src = cst
            else:
                # final pair of the kernel: nobody reuses the slot, skip the
                # copy to shorten the tail chain
                src = ctx_ps
            recb = scr.tile([64, QC], FP32, tag="recb", name=f"rb_{sc}_{h}")
            nc.gpsimd.partition_broadcast(recb[:], rec[:])
            nc.vector.tensor_mul(
                ctxT[mt][r0:r0 + 64, s0:s0 + QC],
                src[0:64, :], recb[:])

        # ---- main schedule ----
        # exp-table warm: first ACT exp pays the ~2.7us table load; issue a
        # tiny one immediately so it overlaps the initial DMA.
        warm = scr.tile([1, 1], FP32, tag="warm", bufs=1, name="warm")
        nc.gpsimd.memset(warm[:], 0.0)
        nc.scalar.activation(warm[:], warm[:], Exp)

        # only chunk 0's mt=0 q/k projections run serially before the first
        # attention wave (DMA-gated anyway); mt=1 and V are pinned inside
        # chunk 0's waves below.
        emit_proj_qk_interleaved(0, 0)
        for pc in proj_v_pieces(0):
            pc()
        pending_norms = []
        for sc in range(NSC):
            nkb = 4 * (sc + 1)
            waves = [(pair, kb) for pair in range(2)
                     for kb in range(nkb)]
            # Filler distribution. Only chunk c's mt=0 q/k projections must
            # strictly precede attention(c) (its pair-0 scores use them at
            # wave 0); mt=1 q/k feed pair 1 (wave nkb) and V feeds ctx from
            # wave 4c, so both are head-pinned INSIDE attention(c) itself.
            # This shifts filler from the early (PE-bound) chunks into the
            # later ACT-paced stretches.
            head = []   # pieces pinned to the earliest waves, one per wave
            extra = []  # pieces distributed evenly over all waves
            pins = {}   # wave -> pieces with exact placement constraints
            if sc == 0:
                # mt=1 q/k is only needed by pair 1's scores at wave 4
                qk0 = proj_qk_pieces(0)
                pins = {0: [qk0[2]], 1: [qk0[3]]}
                extra += proj_qk_pieces(1)[:2]
            elif sc == 1:
                head += proj_qk_pieces(1)[2:] + proj_v_pieces(1)
                extra += proj_qk_pieces(2)[:2]
            elif sc == 2:
                head += proj_qk_pieces(2)[2:] + proj_v_pieces(2)
                extra += proj_qk_pieces(3)[:2] + outproj_pieces(0)
            else:
                head += proj_qk_pieces(3)[2:] + proj_v_pieces(3)
                extra += outproj_pieces(1) + outproj_pieces(2)
            # 'extra' pieces include out-projections that read ctxT, which
            # (coarse tile deps) wait on the previous chunk's normalization
            # chain — starting them a few waves in keeps them off the PE's
            # in-order critical path at the chunk boundary.
            sched = {w: [] for w in range(len(waves))}
            for w, pcs in pins.items():
                sched[w].extend(pcs)
            for j, pc in enumerate(head):
                sched[j].append(pc)
            if extra:
                w0 = min(4, len(waves) - len(extra))
                span_w = len(waves) - w0
                for j, pc in enumerate(extra):
                    sched[w0 + j * span_w // len(extra)].append(pc)

            ctx_tiles = {}
            ctx_queue = []
            for w, (pair, kb) in enumerate(waves):
                if kb == 0:
                    ctx_tiles[pair] = [
                        ps.tile([65, QC], FP32, tag="ctx", bufs=2,
                                name=f"c_{sc}_{pair}_{hh}")
                        for hh in range(2)]
                ex = wave_scores(sc, pair, kb)
                if pending_norms and w in (2, 3):
                    # previous chunk's deferred pair-1 normalization: emitted
                    # a few waves in (not at its own chunk's end) so its
                    # DVE/Pool chain pipelines behind this chunk's first
                    # waves instead of head-of-line-blocking the PE at the
                    # boundary; one head per wave spreads the chain.
                    pending_norms.pop(0)()
                for pc in sched[w]:
                    pc()
                # defer each pair's first two ctx matmuls by two waves: the
                # pair's ctx PSUM slots are recycled from the predecessor
                # pair, whose normalization staging copy needs a couple of
                # waves to execute — emitting ctx immediately would stall
                # the in-order PE on the slot.
                ctx_queue.append((pair, kb, ex))
                lag = 4 if kb < 4 else 0
                while len(ctx_queue) > lag or \
                        (ctx_queue and kb == nkb - 1):
                    qpair, qkb, qex = ctx_queue.pop(0)
                    wave_ctx(sc, qpair, qkb, qex, ctx_tiles[qpair], nkb)
                if kb == nkb - 1:
                    if pair == 1 and sc + 1 < NSC:
                        pending_norms = [
                            (lambda sc=sc, h=2 * pair + hh,
                                    t=ctx_tiles[pair][hh]:
                             emit_norm(sc, h, t))
                            for hh in range(2)]
                    else:
                        last = pair == 1 and sc == NSC - 1
                        for hh in range(2):
                            emit_norm(sc, 2 * pair + hh, ctx_tiles[pair][hh],
                                      stage=not last)
        for pc in outproj_pieces(NSC - 1):
            pc()


def build_module():
    nc = bacc.Bacc("TRN2", target_bir_lowering=False, debug=False)
    xT = nc.dram_tensor("xT", [E, S], BF16, kind="ExternalInput").ap()
    wq = nc.dram_tensor("wq", [E, M], BF16, kind="ExternalInput").ap()
    wk = nc.dram_tensor("wk", [E, M], BF16, kind="ExternalInput").ap()
    wv = nc.dram_tensor("wv", [E, M], BF16, kind="ExternalInput").ap()
    wo = nc.dram_tensor("wo", [M, E], BF16, kind="ExternalInput").ap()
    out = nc.dram_tensor("out", [S, E], FP32, kind="ExternalOutput").ap()
    with tile.TileContext(nc) as tc:
        _emit_kernel(tc, xT, wq, wk, wv, wo, out)
    nc.compile()
    return nc


def make_in_maps(x, w_qkv):
    """Per-core input dicts (bf16, pre-transposed host-side)."""
    bf = ml_dtypes.bfloat16
    xTb = [np.ascontiguousarray(x[b].T).astype(bf) for b in range(B)]
    in_maps = []
    for c in range(NCORES):
        b, g = c // 4, c % 4
        cols = slice(g * M, (g + 1) * M)
        in_maps.append({
            "xT": xTb[b],
            "wq": np.ascontiguousarray(w_qkv[cols, :].T).astype(bf),
            "wk": np.ascontiguousarray(w_qkv[E:][cols, :].T).astype(bf),
            "wv": np.ascontiguousarray(w_qkv[2 * E:][cols, :].T).astype(bf),
            "wo": None,  # filled in kernel(), needs w_out
        })
    return in_maps


_RUNNER = None
_SHARDED = None


def _get_runner():
    """Build the Bass module once and return a cached callable
    (in_maps) -> [NCORES, S, E] fp32 partial outputs."""
    global _RUNNER
    if _RUNNER is not None:
        return _RUNNER

    nc = build_module()

    from concourse import bass2jax
    import jax
    from jax.sharding import Mesh, PartitionSpec
    from jax.experimental.shard_map import shard_map

    bass2jax.install_neuronx_cc_hook()

    in_names = ["xT", "wq", "wk", "wv", "wo"]
    out_names = ["out"]
    out_avals = [jax.core.ShapedArray((S, E), np.float32)]
    n_params = len(in_names)
    all_names = in_names + out_names
    partition_name = (nc.partition_id_tensor.name
                      if nc.partition_id_tensor is not None else None)
    if partition_name is not None:
        all_names = all_names + [partition_name]

    def _body(*args):
        operands = list(args)
        if partition_name is not None:
            operands.append(bass2jax.partition_id_tensor())
        outs = bass2jax._bass_exec_p.bind(
            *operands,
            out_avals=tuple(out_avals),
            in_names=tuple(all_names),
            out_names=tuple(out_names),
            lowering_input_output_aliases=(),
            sim_require_finite=True,
            sim_require_nnan=True,
            nc=nc,
        )
        return tuple(outs)

    devices = jax.devices()[:NCORES]
    mesh = Mesh(np.asarray(devices), ("core",))
    n_outs = len(out_names)
    in_specs = (PartitionSpec("core"),) * (n_params + n_outs)
    out_specs = (PartitionSpec("core"),) * n_outs
    sharded = jax.jit(
        shard_map(_body, mesh=mesh, in_specs=in_specs, out_specs=out_specs,
                  check_rep=False),
        donate_argnums=tuple(range(n_params, n_params + n_outs)),
        keep_unused=True,
    )
    global _SHARDED
    _SHARDED = sharded

    def run(in_maps):
        concat_in = [
            np.concatenate([np.asarray(in_maps[c][n]) for c in range(NCORES)],
                           axis=0)
            for n in in_names
        ]
        concat_zeros = [np.zeros((NCORES * S, E), np.float32)]
        out_arrs = sharded(*concat_in, *concat_zeros)
        return np.asarray(out_arrs[0]).reshape(NCORES, S, E)

    _RUNNER = run
    return run


def kernel(x, w_qkv, w_out, b_out):
    x = np.asarray(x, dtype=np.float32)
    w_qkv = np.asarray(w_qkv, dtype=np.float32)
    w_out = np.asarray(w_out, dtype=np.float32)
    b_out = np.asarray(b_out, dtype=np.float32)

    bf = ml_dtypes.bfloat16
    in_maps = make_in_maps(x, w_qkv)
    for c in range(NCORES):
        g = c % 4
        cols = slice(g * M, (g + 1) * M)
        in_maps[c]["wo"] = np.ascontiguousarray(w_out[:, cols].T).astype(bf)

    run = _get_runner()
    partials = run(in_maps)  # [8, S, E] fp32

    out = np.empty((B, S, E), np.float32)
    for b in range(B):
        acc = partials[4 * b].astype(np.float64)
        for i in range(1, 4):
            acc += partials[4 * b + i]
        out[b] = (acc + b_out.astype(np.float64)).astype(np.float32)
    return out



# revision 2
# speedup vs baseline: 1.1404x; 1.1404x over previous
"""Multi-head causal self-attention (B=2, S=2048, E=1024, H=16, D=64) on 8
Trainium2 NeuronCores.

Sharding: batch x head-group. Core c handles batch (c // 4) and heads
[4*(c%4), 4*(c%4)+4). Each core computes QKV projection for its 4 heads,
causal flash-attention, and a partial output projection over its head
columns. Host sums the 4 partial outputs per batch and adds b_out.

v3 changes vs the original baseline:
  - warmup dummy matmuls ride out the DMA-gated startup so the PE p-state
    ramp (0.65 -> 1.2 -> 2.4 GHz) completes before real work arrives
  - normalization reads ctx PSUM directly (no staging copy); ctx psum slot
    slack comes from the deferred-ctx lag
  - proj/outproj PSUM gets bufs=2 (removes mm->copy serialization)
  - rel=1 score blocks skip their fully-masked first 128 columns
  - partial outputs stored bf16 (halves store DMA), host sums in fp64
  - tail outproj PSUM reuses the (idle by then) scores banks
"""

import sys

if "/opt/trn_rl_repo" not in sys.path:
    sys.path.insert(0, "/opt/trn_rl_repo")

import numpy as np
import ml_dtypes

import concourse.bacc as bacc
import concourse.mybir as mybir
import concourse.tile as tile

BF16 = mybir.dt.bfloat16
FP32 = mybir.dt.float32

B, S, E = 2, 2048, 1024
H, DH = 16, 64
NCORES = 8
HPC = 4            # heads per core
M = HPC * DH       # 256 ctx columns per core
QC = 512           # q chunk (max wave width; also PSUM head stride)
KB = 128           # k block
SCALE = 1.0 / np.sqrt(DH)
NWARM = 64         # warmup dummy matmuls (128 cols each)
# q-chunks (q0, Q, grp). The last 512 splits 384+128 so the final
# norm/outproj tail is 4x smaller. grp = k-blocks per wave: the narrow final
# chunk processes 4 k-blocks per scores-PSUM tile / exp call, so its waves
# are not paced by per-call ACT overhead.
CHUNKS = [(0, 512, 1), (512, 512, 1), (1024, 512, 1),
          (1536, 384, 1), (1920, 128, 4)]
NCH = len(CHUNKS)


def _emit_kernel(tc, xT, wq, wk, wv, wo_d, out):
    nc = tc.nc
    Exp = mybir.ActivationFunctionType.Exp

    with tc.tile_pool(name="res", bufs=1) as res, \
         tc.tile_pool(name="ps", bufs=1, space="PSUM") as ps, \
         tc.tile_pool(name="expp", bufs=6) as expp, \
         tc.tile_pool(name="scr", bufs=4) as scr, \
         tc.tile_pool(name="outb", bufs=2) as outb:

        # ---- resident SBUF tiles ----
        xt_all = res.tile([128, 8 * S], BF16, name="xt_all")
        xt = [xt_all[:, e * S:(e + 1) * S] for e in range(8)]
        xt_3d = xt_all.rearrange("p (e s) -> p e s", s=S)
        wqt = res.tile([128, 8 * M], BF16, name="wqt")
        wkt = res.tile([128, 8 * M], BF16, name="wkt")
        wvt = res.tile([128, 8 * M], BF16, name="wvt")
        wot = [res.tile([128, E], BF16, name=f"wot{i}") for i in range(2)]
        qTt = [res.tile([128, S], BF16, name=f"qTt{i}") for i in range(2)]
        kTt = [res.tile([128, S], BF16, name=f"kTt{i}") for i in range(2)]
        ctxT = [res.tile([128, S], BF16, name=f"ctxT{i}") for i in range(2)]
        # V with ones column: per (k-block kb, head h) a [128, 65] slab
        v1 = res.tile([128, (S // KB) * HPC * 65], BF16, name="v1")
        v1_3d = v1.rearrange("p (n c) -> p n c", c=65)
        mask = res.tile([128, 128], BF16, name="mask")
        warm_src = res.tile([128, 128], BF16, name="warm_src")

        # ---- warmup: dummy matmuls keep the PE busy (and its p-state
        # ramping) through the DMA-gated startup. They read a memset tile and
        # write a throwaway PSUM slot; the first real matmul enters a fully
        # ramped (2.4 GHz) engine.
        nc.gpsimd.memset(warm_src[:], 0.0)
        warm_ps = ps.tile([128, QC], FP32, tag="proj", bufs=2, name="warm_ps")
        for i in range(NWARM):
            nc.tensor.matmul(
                warm_ps[:, 0:128], lhsT=warm_src[:], rhs=warm_src[:],
                start=True, stop=True)

        # ---- input DMA: one batched transfer per tensor/chunk ----
        wqt_3d = wqt.rearrange("p (e m) -> p e m", m=M)
        wkt_3d = wkt.rearrange("p (e m) -> p e m", m=M)
        wvt_3d = wvt.rearrange("p (e m) -> p e m", m=M)
        xT_3d = xT.rearrange("(e p) s -> p e s", p=128)
        nc.sync.dma_start(wqt_3d[:], wq.rearrange("(e p) m -> p e m", p=128))
        nc.sync.dma_start(xt_3d[:, 0:4, 0:QC], xT_3d[:, 0:4, 0:QC])
        nc.sync.dma_start(wkt_3d[:], wk.rearrange("(e p) m -> p e m", p=128))
        nc.sync.dma_start(xt_3d[:, 4:8, 0:QC], xT_3d[:, 4:8, 0:QC])
        nc.sync.dma_start(wvt_3d[:], wv.rearrange("(e p) m -> p e m", p=128))
        for chunk in range(1, S // QC):
            nc.sync.dma_start(
                xt_3d[:, :, chunk * QC:(chunk + 1) * QC],
                xT_3d[:, :, chunk * QC:(chunk + 1) * QC])
        for i in range(2):
            nc.sync.dma_start(wot[i][:], wo_d[i * 128:(i + 1) * 128, :])

        # ---- constants ----
        nc.gpsimd.memset(v1[:], 1.0)  # data columns overwritten by V proj
        # stair mask: keep where k_local <= q_local (within a 128x128 block)
        nc.gpsimd.memset(mask[:], 1.0)
        nc.gpsimd.affine_select(
            out=mask[:], in_=mask[:],
            compare_op=mybir.AluOpType.is_ge,
            fill=0.0, base=0,
            pattern=[[1, 128]],
            channel_multiplier=-1,
        )

        # ---- emission helpers ----
        def emit_proj_qk(ci, mt, wt, dstt, kind):
            s0, Q, _ = CHUNKS[ci]
            pqk = ps.tile([128, QC], FP32, tag="proj", bufs=2,
                          name=f"p{kind}_{ci}_{mt}")
            for e in range(8):
                nc.tensor.matmul(
                    pqk[:, 0:Q],
                    lhsT=wt[:, e * M + mt * 128: e * M + (mt + 1) * 128],
                    rhs=xt[e][:, s0:s0 + Q],
                    start=(e == 0), stop=(e == 7))
            nc.vector.tensor_copy(dstt[mt][:, s0:s0 + Q], pqk[:, 0:Q])

        def emit_proj_qk_interleaved(ci, mt):
            # q and k chains interleaved per e-chunk for the DMA-gated start
            s0, Q, _ = CHUNKS[ci]
            pq = ps.tile([128, QC], FP32, tag="proj", bufs=2,
                         name=f"pqi_{ci}_{mt}")
            pk = ps.tile([128, QC], FP32, tag="proj", bufs=2,
                         name=f"pki_{ci}_{mt}")
            for e in range(8):
                nc.tensor.matmul(
                    pq[:, 0:Q],
                    lhsT=wqt[:, e * M + mt * 128: e * M + (mt + 1) * 128],
                    rhs=xt[e][:, s0:s0 + Q],
                    start=(e == 0), stop=(e == 7))
                nc.tensor.matmul(
                    pk[:, 0:Q],
                    lhsT=wkt[:, e * M + mt * 128: e * M + (mt + 1) * 128],
                    rhs=xt[e][:, s0:s0 + Q],
                    start=(e == 0), stop=(e == 7))
            nc.vector.tensor_copy(qTt[mt][:, s0:s0 + Q], pq[:, 0:Q])
            nc.vector.tensor_copy(kTt[mt][:, s0:s0 + Q], pk[:, 0:Q])

        def emit_proj_v(sblk):
            pv = ps.tile([128, M], FP32, tag="proj", bufs=2, name=f"pv_{sblk}")
            for e in range(8):
                nc.tensor.matmul(
                    pv[:],
                    lhsT=xt[e][:, sblk * 128:(sblk + 1) * 128],
                    rhs=wvt[:, e * M:(e + 1) * M],
                    start=(e == 0), stop=(e == 7))
            nc.vector.tensor_copy(
                v1_3d[:, sblk * HPC:(sblk + 1) * HPC, 0:64],
                pv[:].rearrange("p (h c) -> p h c", c=64))

        def proj_qk_pieces(ci):
            pcs = []
            for mt in range(2):
                pcs.append(lambda mt=mt: emit_proj_qk(ci, mt, wqt, qTt, "q"))
                pcs.append(lambda mt=mt: emit_proj_qk(ci, mt, wkt, kTt, "k"))
            return pcs

        def proj_v_pieces(blks):
            return [lambda sb=sb: emit_proj_v(sb) for sb in blks]

        ob_tiles = {}
        out_3d = out.rearrange("(q p) f -> p q f", p=128)

        def emit_outproj(ci, qq, fc, tail=False):
            q0, Q, _ = CHUNKS[ci]
            nqb = Q // 128
            qb = q0 // 128 + qq
            last = ci == NCH - 1
            if qq == 0 and fc == 0:
                ob_tiles[ci] = outb.tile([128, nqb * E], BF16, tag="ob",
                                         name=f"ob_{qb}")
            ob = ob_tiles[ci]
            # tail outprojs borrow the scores PSUM slots (attention is done
            # by then), keeping mm->copy->mm free of slot serialization
            tag = "scores" if tail else "proj"
            po = ps.tile([128, QC], FP32, tag=tag, bufs=2,
                         name=f"po_{qb}_{fc}")
            for mc in range(2):
                nc.tensor.matmul(
                    po[:],
                    lhsT=ctxT[mc][:, qb * 128:(qb + 1) * 128],
                    rhs=wot[mc][:, fc * QC:(fc + 1) * QC],
                    start=(mc == 0), stop=(mc == 1))
            nc.vector.tensor_copy(
                ob[:, qq * E + fc * QC: qq * E + (fc + 1) * QC], po[:])
            if last:
                # final chunk: store each fc half as soon as it's staged so
                # the exposed end-of-kernel DMA is a single small transfer
                nc.sync.dma_start(
                    out[qb * 128:(qb + 1) * 128, fc * QC:(fc + 1) * QC],
                    ob[:, qq * E + fc * QC: qq * E + (fc + 1) * QC])
                if (qq, fc) == (nqb - 1, 1):
                    del ob_tiles[ci]
            elif (qq, fc) == (nqb - 1, 1):
                nc.sync.dma_start(
                    out_3d[:, q0 // 128: q0 // 128 + nqb, :],
                    ob.rearrange("p (q f) -> p q f", f=E))
                del ob_tiles[ci]

        def outproj_pieces(ci, tail=False):
            _, Q, _ = CHUNKS[ci]
            return [lambda qq=qq, fc=fc: emit_outproj(ci, qq, fc, tail=tail)
                    for qq in range(Q // 128) for fc in range(2)]

        # ---- attention waves (one head PAIR, grp k-blocks) ----
        # kd = kb*128 - q0: offset of the k-block's diagonal within the
        # chunk's q columns. kd >= 128: cols [0, kd) are fully masked -> skip
        # in scores (exp still covers them for kd == 128; the garbage is
        # never consumed). kd >= 0: stair-mask cols [kd, kd+128).
        # For grp > 1, each wave covers grp consecutive k-blocks laid out as
        # column groups of width Q inside the head's PSUM half, sharing one
        # exp call.
        def wave_scores(ci, pair, g):
            s0, Q, grp = CHUNKS[ci]
            mt = pair
            sc_ps = ps.tile([128, 2 * QC], FP32, tag="scores", bufs=2,
                            name=f"s_{ci}_{pair}_{g}")
            kds = [(j, (g * grp + j) * 128 - s0) for j in range(grp)]
            lo_e = 0
            for hh in range(2):
                r0 = hh * 64
                off = hh * QC
                for j, kd in kds:
                    kb = g * grp + j
                    lo = kd if (kd >= 128 and grp == 1) else 0
                    if hh == 0 and kd >= 256 and grp == 1:
                        lo_e = kd
                    nc.tensor.matmul(
                        sc_ps[:, off + j * Q + lo: off + (j + 1) * Q],
                        lhsT=kTt[mt][r0:r0 + 64, kb * 128:(kb + 1) * 128],
                        rhs=qTt[mt][r0:r0 + 64, s0 + lo: s0 + Q],
                        start=True, stop=True)
            ex = expp.tile([128, 2 * QC], BF16, tag="ex",
                           name=f"e_{ci}_{pair}_{g}")
            W = grp * Q
            if lo_e or W < QC:
                # both heads in one strided-AP call: the ACT engine charges
                # by total free size, so this halves the per-call init cost
                # vs one call per head
                ex3 = ex.rearrange("p (h q) -> p h q", h=2)
                sc3 = sc_ps.rearrange("p (h q) -> p h q", h=2)
                nc.scalar.activation(ex3[:, :, lo_e:W], sc3[:, :, lo_e:W],
                                     Exp, scale=SCALE)
            else:
                nc.scalar.activation(ex[:], sc_ps[:], Exp, scale=SCALE)
            for hh in range(2):
                off = hh * QC
                for j, kd in kds:
                    if kd >= 0:
                        nc.vector.tensor_mul(
                            ex[:, off + j * Q + kd: off + j * Q + kd + 128],
                            ex[:, off + j * Q + kd: off + j * Q + kd + 128],
                            mask[:])
            return ex

        def wave_ctx(ci, pair, g, ex, ctx_pair, nkb):
            s0, Q, grp = CHUNKS[ci]
            for hh in range(2):
                h = 2 * pair + hh
                off = hh * QC
                for j in range(grp):
                    kb = g * grp + j
                    kd = kb * 128 - s0
                    lo = max(kd, 0)
                    nc.tensor.matmul(
                        ctx_pair[hh][:, lo:Q],
                        lhsT=v1_3d[:, kb * HPC + h, :],
                        rhs=ex[:, off + j * Q + lo: off + (j + 1) * Q],
                        start=(kb == 0), stop=(kb == nkb - 1),
                        skip_group_check=True)

        def emit_norm(ci, h, ctx_ps):
            s0, Q, _ = CHUNKS[ci]
            mt, r0 = h // 2, (h % 2) * 64
            # recip reads the PSUM denom row; broadcast on Pool; the norm
            # multiply reads ctx PSUM directly (no staging copy) and writes
            # bf16 ctxT. The ctx PSUM slot is released when the multiply
            # completes -- the next same-parity pair's deferred first ctx
            # matmuls give it the needed slack.
            rec = scr.tile([1, QC], FP32, tag="rec", name=f"r_{ci}_{h}")
            nc.vector.reciprocal(rec[:, 0:Q], ctx_ps[64:65, 0:Q])
            recb = scr.tile([64, QC], FP32, tag="recb", name=f"rb_{ci}_{h}")
            nc.gpsimd.partition_broadcast(recb[:, 0:Q], rec[:, 0:Q])
            nc.vector.tensor_mul(
                ctxT[mt][r0:r0 + 64, s0:s0 + Q],
                ctx_ps[0:64, 0:Q], recb[:, 0:Q])

        def emit_norm_pair(ci, items):
            # final-pair norms: interleave the two heads' recip/broadcast/
            # multiply so the DVE and Pool stages pipeline instead of
            # serializing head-by-head at the kernel tail
            s0, Q, _ = CHUNKS[ci]
            recs = []
            for h, ctx_ps in items:
                rec = scr.tile([1, QC], FP32, tag="rec", name=f"r_{ci}_{h}")
                nc.vector.reciprocal(rec[:, 0:Q], ctx_ps[64:65, 0:Q])
                recs.append(rec)
            recbs = []
            for (h, _), rec in zip(items, recs):
                recb = scr.tile([64, QC], FP32, tag="recb",
                                name=f"rb_{ci}_{h}")
                nc.gpsimd.partition_broadcast(recb[:, 0:Q], rec[:, 0:Q])
                recbs.append(recb)
            for (h, ctx_ps), recb in zip(items, recbs):
                mt, r0 = h // 2, (h % 2) * 64
                nc.vector.tensor_mul(
                    ctxT[mt][r0:r0 + 64, s0:s0 + Q],
                    ctx_ps[0:64, 0:Q], recb[:, 0:Q])

        # ---- main schedule ----
        # exp-table warm: overlap the ~2.7us table load with the initial DMA
        warm = scr.tile([1, 1], FP32, tag="warm", bufs=1, name="warm")
        nc.gpsimd.memset(warm[:], 0.0)
        nc.scalar.activation(warm[:], warm[:], Exp)

        emit_proj_qk_interleaved(0, 0)
        pending_norms = []
        for ci in range(NCH):
            q0, Q, grp = CHUNKS[ci]
            nkb = (q0 + Q) // 128
            ngrp = nkb // grp
            waves = [(pair, g) for pair in range(2)
                     for g in range(ngrp)]
            head = []   # pieces pinned to the earliest waves, one per wave
            extra = []  # pieces distributed evenly over all waves
            pins = {}   # wave -> pieces with exact placement constraints
            if ci == 0:
                qk0 = proj_qk_pieces(0)
                v0 = proj_v_pieces(range(0, 4))
                pins = {0: [qk0[2], v0[0]], 1: [qk0[3], v0[1]],
                        2: [v0[2]], 3: [v0[3]]}
                extra += proj_qk_pieces(1)[:2]
            elif ci == 1:
                head += proj_qk_pieces(1)[2:] + proj_v_pieces(range(4, 8))
                extra += proj_qk_pieces(2)[:2]
            elif ci == 2:
                head += proj_qk_pieces(2)[2:] + proj_v_pieces(range(8, 12))
                extra += proj_qk_pieces(3)[:2] + outproj_pieces(0)
            elif ci == 3:
                head += (proj_qk_pieces(3)[2:] + proj_qk_pieces(4)
                         + proj_v_pieces(range(12, 16)))
                extra += outproj_pieces(1)
            else:
                extra += outproj_pieces(2) + outproj_pieces(3)
            sched = {w: [] for w in range(len(waves))}
            for w, pcs in pins.items():
                sched[w].extend(pcs)
            for j, pc in enumerate(head):
                sched[j].append(pc)
            if extra:
                w0 = max(0, min(4, len(waves) - len(extra)))
                span_w = len(waves) - w0
                for j, pc in enumerate(extra):
                    sched[w0 + j * span_w // len(extra)].append(pc)

            ctx_tiles = {}
            ctx_queue = []
            for w, (pair, g) in enumerate(waves):
                if g == 0:
                    ctx_tiles[pair] = [
                        ps.tile([65, QC], FP32, tag="ctx", bufs=2,
                                name=f"c_{ci}_{pair}_{hh}")
                        for hh in range(2)]
                ex = wave_scores(ci, pair, g)
                if pending_norms and w in (2, 3):
                    pending_norms.pop(0)()
                for pc in sched[w]:
                    pc()
                ctx_queue.append((pair, g, ex))
                lag = max(1, 4 // grp) if g * grp < 4 else 0
                while len(ctx_queue) > lag or \
                        (ctx_queue and g == ngrp - 1):
                    qpair, qg, qex = ctx_queue.pop(0)
                    wave_ctx(ci, qpair, qg, qex, ctx_tiles[qpair], nkb)
                if g == ngrp - 1:
                    if pair == 1 and ci + 1 < NCH:
                        pending_norms = [
                            (lambda ci=ci, h=2 * pair + hh,
                                    t=ctx_tiles[pair][hh]:
                             emit_norm(ci, h, t))
                            for hh in range(2)]
                    elif pair == 1:
                        emit_norm_pair(ci, [(2 + hh, ctx_tiles[pair][hh])
                                            for hh in range(2)])
                    else:
                        emit_norm_pair(ci, [(hh, ctx_tiles[pair][hh])
                                            for hh in range(2)])
        for pc in outproj_pieces(NCH - 1, tail=True):
            pc()


def build_module():
    nc = bacc.Bacc("TRN2", target_bir_lowering=False, debug=False)
    xT = nc.dram_tensor("xT", [E, S], BF16, kind="ExternalInput").ap()
    wq = nc.dram_tensor("wq", [E, M], BF16, kind="ExternalInput").ap()
    wk = nc.dram_tensor("wk", [E, M], BF16, kind="ExternalInput").ap()
    wv = nc.dram_tensor("wv", [E, M], BF16, kind="ExternalInput").ap()
    wo = nc.dram_tensor("wo", [M, E], BF16, kind="ExternalInput").ap()
    out = nc.dram_tensor("out", [S, E], BF16, kind="ExternalOutput").ap()
    with tile.TileContext(nc) as tc:
        _emit_kernel(tc, xT, wq, wk, wv, wo, out)
    nc.compile()
    return nc


def make_in_maps(x, w_qkv):
    """Per-core input dicts (bf16, pre-transposed host-side)."""
    bf = ml_dtypes.bfloat16
    xTb = [np.ascontiguousarray(x[b].T).astype(bf) for b in range(B)]
    in_maps = []
    for c in range(NCORES):
        b, g = c // 4, c % 4
        cols = slice(g * M, (g + 1) * M)
        in_maps.append({
            "xT": xTb[b],
            "wq": np.ascontiguousarray(w_qkv[cols, :].T).astype(bf),
            "wk": np.ascontiguousarray(w_qkv[E:][cols, :].T).astype(bf),
            "wv": np.ascontiguousarray(w_qkv[2 * E:][cols, :].T).astype(bf),
            "wo": None,  # filled in kernel(), needs w_out
        })
    return in_maps


_RUNNER = None
_SHARDED = None


def _get_runner():
    """Build the Bass module once and return a cached callable
    (in_maps) -> [NCORES, S, E] bf16 partial outputs."""
    global _RUNNER
    if _RUNNER is not None:
        return _RUNNER

    nc = build_module()

    from concourse import bass2jax
    import jax
    from jax.sharding import Mesh, PartitionSpec
    from jax.experimental.shard_map import shard_map

    bass2jax.install_neuronx_cc_hook()

    in_names = ["xT", "wq", "wk", "wv", "wo"]
    out_names = ["out"]
    out_avals = [jax.core.ShapedArray((S, E), ml_dtypes.bfloat16)]
    n_params = len(in_names)
    all_names = in_names + out_names
    partition_name = (nc.partition_id_tensor.name
                      if nc.partition_id_tensor is not None else None)
    if partition_name is not None:
        all_names = all_names + [partition_name]

    def _body(*args):
        operands = list(args)
        if partition_name is not None:
            operands.append(bass2jax.partition_id_tensor())
        outs = bass2jax._bass_exec_p.bind(
            *operands,
            out_avals=tuple(out_avals),
            in_names=tuple(all_names),
            out_names=tuple(out_names),
            lowering_input_output_aliases=(),
            sim_require_finite=True,
            sim_require_nnan=True,
            nc=nc,
        )
        return tuple(outs)

    devices = jax.devices()[:NCORES]
    mesh = Mesh(np.asarray(devices), ("core",))
    n_outs = len(out_names)
    in_specs = (PartitionSpec("core"),) * (n_params + n_outs)
    out_specs = (PartitionSpec("core"),) * n_outs
    sharded = jax.jit(
        shard_map(_body, mesh=mesh, in_specs=in_specs, out_specs=out_specs,
                  check_rep=False),
        donate_argnums=tuple(range(n_params, n_params + n_outs)),
        keep_unused=True,
    )
    global _SHARDED
    _SHARDED = sharded

    def run(in_maps):
        concat_in = [
            np.concatenate([np.asarray(in_maps[c][n]) for c in range(NCORES)],
                           axis=0)
            for n in in_names
        ]
        concat_zeros = [np.zeros((NCORES * S, E), ml_dtypes.bfloat16)]
        out_arrs = sharded(*concat_in, *concat_zeros)
        return np.asarray(out_arrs[0]).reshape(NCORES, S, E)

    _RUNNER = run
    return run


def kernel(x, w_qkv, w_out, b_out):
    x = np.asarray(x, dtype=np.float32)
    w_qkv = np.asarray(w_qkv, dtype=np.float32)
    w_out = np.asarray(w_out, dtype=np.float32)
    b_out = np.asarray(b_out, dtype=np.float32)

    bf = ml_dtypes.bfloat16
    in_maps = make_in_maps(x, w_qkv)
    for c in range(NCORES):
        g = c % 4
        cols = slice(g * M, (g + 1) * M)
        in_maps[c]["wo"] = np.ascontiguousarray(w_out[:, cols].T).astype(bf)

    run = _get_runner()
    partials = run(in_maps)  # [8, S, E] bf16

    out = np.empty((B, S, E), np.float32)
    for b in range(B):
        acc = partials[4 * b].astype(np.float64)
        for i in range(1, 4):
            acc += partials[4 * b + i].astype(np.float64)
        out[b] = (acc + b_out.astype(np.float64)).astype(np.float32)
    return out


# revision 3
# speedup vs baseline: 1.2108x; 1.0618x over previous
"""Multi-head causal self-attention (B=2, S=2048, E=1024, H=16, D=64) on 8
Trainium2 NeuronCores.

Sharding: batch x head-group. Core c handles batch (c // 4) and heads
[4*(c%4), 4*(c%4)+4). Each core computes QKV projection for its 4 heads,
causal flash-attention, and a partial output projection over its head
columns. Host sums the 4 partial outputs per batch and adds b_out.

v3 changes vs the original baseline:
  - warmup dummy matmuls ride out the DMA-gated startup so the PE p-state
    ramp (0.65 -> 1.2 -> 2.4 GHz) completes before real work arrives
  - normalization reads ctx PSUM directly (no staging copy); ctx psum slot
    slack comes from the deferred-ctx lag
  - proj/outproj PSUM gets bufs=2 (removes mm->copy serialization)
  - rel=1 score blocks skip their fully-masked first 128 columns
  - partial outputs stored bf16 (halves store DMA), host sums in fp64
  - tail outproj PSUM reuses the (idle by then) scores banks
"""

import sys

if "/opt/trn_rl_repo" not in sys.path:
    sys.path.insert(0, "/opt/trn_rl_repo")

import numpy as np
import ml_dtypes

import concourse.bacc as bacc
import concourse.mybir as mybir
import concourse.tile as tile

BF16 = mybir.dt.bfloat16
FP32 = mybir.dt.float32
F8 = mybir.dt.float8e4
DRMODE = mybir.MatmulPerfMode.DoubleRow

B, S, E = 2, 2048, 1024
H, DH = 16, 64
NCORES = 8
HPC = 4            # heads per core
M = HPC * DH       # 256 ctx columns per core
QC = 512           # q chunk (max wave width; also PSUM head stride)
KB = 128           # k block
SCALE = 1.0 / np.sqrt(DH)
NWARM = 64         # warmup dummy matmuls (128 cols each)
# q-chunks (q0, Q, grp). The last 512 splits 384+128 so the final
# norm/outproj tail is 4x smaller. grp = k-blocks per wave: the narrow final
# chunk processes 4 k-blocks per scores-PSUM tile / exp call, so its waves
# are not paced by per-call ACT overhead.
CHUNKS = [(0, 512, 1), (512, 512, 1), (1024, 512, 1),
          (1536, 384, 1), (1920, 128, 4)]
NCH = len(CHUNKS)


def _emit_kernel(tc, xT, wq, wk, wv, wo_d, out):
    nc = tc.nc
    Exp = mybir.ActivationFunctionType.Exp

    with tc.tile_pool(name="res", bufs=1) as res, \
         tc.tile_pool(name="ps", bufs=1, space="PSUM") as ps, \
         tc.tile_pool(name="expp", bufs=6) as expp, \
         tc.tile_pool(name="scr", bufs=4) as scr, \
         tc.tile_pool(name="outb", bufs=2) as outb:

        # ---- resident SBUF tiles ----
        xt_all = res.tile([128, 8 * S], BF16, name="xt_all")
        xt = [xt_all[:, e * S:(e + 1) * S] for e in range(8)]
        xt_3d = xt_all.rearrange("p (e s) -> p e s", s=S)
        wqt = res.tile([128, 8 * M], BF16, name="wqt")
        wkt = res.tile([128, 8 * M], BF16, name="wkt")
        wvt = res.tile([128, 8 * M], BF16, name="wvt")
        wot = [res.tile([128, E], BF16, name=f"wot{i}") for i in range(2)]
        qTt = [res.tile([128, S], BF16, name=f"qTt{i}") for i in range(2)]
        kTt = [res.tile([128, S], BF16, name=f"kTt{i}") for i in range(2)]
        ctxT = [res.tile([128, S], BF16, name=f"ctxT{i}") for i in range(2)]
        # fp8 scores path (queries >= 512): classic-layout fp8 staging plus
        # DoubleRow "pair" tiles [64, 2*S] with head parity on partition
        # halves {0,32} and the two dh-32 k-tiles side by side in free dim
        q8c = [res.tile([128, S], F8, name=f"q8c{i}") for i in range(2)]
        k8c = [res.tile([128, S], F8, name=f"k8c{i}") for i in range(2)]
        q8p = [res.tile([64, 2 * S], F8, name=f"q8p{i}") for i in range(2)]
        k8p = [res.tile([64, 2 * S], F8, name=f"k8p{i}") for i in range(2)]
        q8p3 = [t.rearrange("p (j s) -> p j s", j=2) for t in q8p]
        k8p3 = [t.rearrange("p (j s) -> p j s", j=2) for t in k8p]
        # V with ones column: per (k-block kb, head h) a [128, 65] slab
        v1 = res.tile([128, (S // KB) * HPC * 65], BF16, name="v1")
        v1_3d = v1.rearrange("p (n c) -> p n c", c=65)
        mask = res.tile([128, 128], BF16, name="mask")
        warm_src = res.tile([128, 128], BF16, name="warm_src")

        # ---- warmup: dummy matmuls keep the PE busy (and its p-state
        # ramping) through the DMA-gated startup. They read a memset tile and
        # write a throwaway PSUM slot; the first real matmul enters a fully
        # ramped (2.4 GHz) engine.
        nc.gpsimd.memset(warm_src[:], 0.0)
        warm_ps = ps.tile([128, QC], FP32, tag="proj", bufs=2, name="warm_ps")
        for i in range(NWARM):
            nc.tensor.matmul(
                warm_ps[:, 0:128], lhsT=warm_src[:], rhs=warm_src[:],
                start=True, stop=True)

        # ---- input DMA: one batched transfer per tensor/chunk ----
        wqt_3d = wqt.rearrange("p (e m) -> p e m", m=M)
        wkt_3d = wkt.rearrange("p (e m) -> p e m", m=M)
        wvt_3d = wvt.rearrange("p (e m) -> p e m", m=M)
        xT_3d = xT.rearrange("(e p) s -> p e s", p=128)
        nc.sync.dma_start(wqt_3d[:], wq.rearrange("(e p) m -> p e m", p=128))
        nc.sync.dma_start(xt_3d[:, 0:4, 0:QC], xT_3d[:, 0:4, 0:QC])
        nc.sync.dma_start(wkt_3d[:], wk.rearrange("(e p) m -> p e m", p=128))
        nc.sync.dma_start(xt_3d[:, 4:8, 0:QC], xT_3d[:, 4:8, 0:QC])
        nc.sync.dma_start(wvt_3d[:], wv.rearrange("(e p) m -> p e m", p=128))
        for chunk in range(1, S // QC):
            nc.sync.dma_start(
                xt_3d[:, :, chunk * QC:(chunk + 1) * QC],
                xT_3d[:, :, chunk * QC:(chunk + 1) * QC])
        for i in range(2):
            nc.sync.dma_start(wot[i][:], wo_d[i * 128:(i + 1) * 128, :])

        # ---- constants ----
        nc.gpsimd.memset(v1[:], 1.0)  # data columns overwritten by V proj
        # stair mask: keep where k_local <= q_local (within a 128x128 block)
        nc.gpsimd.memset(mask[:], 1.0)
        nc.gpsimd.affine_select(
            out=mask[:], in_=mask[:],
            compare_op=mybir.AluOpType.is_ge,
            fill=0.0, base=0,
            pattern=[[1, 128]],
            channel_multiplier=-1,
        )

        # ---- emission helpers ----
        def stage_qk(ci, mt, kind, pqk):
            # chunk 0 queries score in bf16 (classic layout); all other
            # queries score in fp8 DoubleRow. k is needed in fp8 by every
            # fp8 chunk, and in bf16 only for chunk 0's k-blocks.
            s0, Q, _ = CHUNKS[ci]
            c8 = q8c if kind == "q" else k8c
            p8 = q8p3 if kind == "q" else k8p3
            dstt = qTt if kind == "q" else kTt
            if ci == 0:
                nc.vector.tensor_copy(dstt[mt][:, s0:s0 + Q], pqk[:, 0:Q])
            if kind == "k" or ci >= 1:
                nc.vector.tensor_copy(c8[mt][:, s0:s0 + Q], pqk[:, 0:Q])
                # partition rearrange into the DoubleRow pair tile:
                # pt[32*hh + d%32, (d//32)*S + s] = classic[64*hh + d, s]
                for hh in range(2):
                    for j in range(2):
                        nc.sync.dma_start(
                            p8[mt][32 * hh:32 * hh + 32, j, s0:s0 + Q],
                            c8[mt][64 * hh + 32 * j: 64 * hh + 32 * j + 32,
                                   s0:s0 + Q])

        def emit_proj_qk(ci, mt, wt, kind):
            s0, Q, _ = CHUNKS[ci]
            pqk = ps.tile([128, QC], FP32, tag="proj", bufs=2,
                          name=f"p{kind}_{ci}_{mt}")
            for e in range(8):
                nc.tensor.matmul(
                    pqk[:, 0:Q],
                    lhsT=wt[:, e * M + mt * 128: e * M + (mt + 1) * 128],
                    rhs=xt[e][:, s0:s0 + Q],
                    start=(e == 0), stop=(e == 7))
            stage_qk(ci, mt, kind, pqk)

        def emit_proj_qk_interleaved(ci, mt):
            # q and k chains interleaved per e-chunk for the DMA-gated start
            s0, Q, _ = CHUNKS[ci]
            pq = ps.tile([128, QC], FP32, tag="proj", bufs=2,
                         name=f"pqi_{ci}_{mt}")
            pk = ps.tile([128, QC], FP32, tag="proj", bufs=2,
                         name=f"pki_{ci}_{mt}")
            for e in range(8):
                nc.tensor.matmul(
                    pq[:, 0:Q],
                    lhsT=wqt[:, e * M + mt * 128: e * M + (mt + 1) * 128],
                    rhs=xt[e][:, s0:s0 + Q],
                    start=(e == 0), stop=(e == 7))
                nc.tensor.matmul(
                    pk[:, 0:Q],
                    lhsT=wkt[:, e * M + mt * 128: e * M + (mt + 1) * 128],
                    rhs=xt[e][:, s0:s0 + Q],
                    start=(e == 0), stop=(e == 7))
            stage_qk(ci, mt, "q", pq)
            stage_qk(ci, mt, "k", pk)

        def emit_proj_v(sblk):
            pv = ps.tile([128, M], FP32, tag="proj", bufs=2, name=f"pv_{sblk}")
            for e in range(8):
                nc.tensor.matmul(
                    pv[:],
                    lhsT=xt[e][:, sblk * 128:(sblk + 1) * 128],
                    rhs=wvt[:, e * M:(e + 1) * M],
                    start=(e == 0), stop=(e == 7))
            nc.vector.tensor_copy(
                v1_3d[:, sblk * HPC:(sblk + 1) * HPC, 0:64],
                pv[:].rearrange("p (h c) -> p h c", c=64))

        def proj_qk_pieces(ci):
            pcs = []
            for mt in range(2):
                pcs.append(lambda mt=mt: emit_proj_qk(ci, mt, wqt, "q"))
                pcs.append(lambda mt=mt: emit_proj_qk(ci, mt, wkt, "k"))
            return pcs

        def proj_v_pieces(blks):
            return [lambda sb=sb: emit_proj_v(sb) for sb in blks]

        ob_tiles = {}
        out_3d = out.rearrange("(q p) f -> p q f", p=128)

        def emit_outproj(ci, qq, fc, tail=False):
            q0, Q, _ = CHUNKS[ci]
            nqb = Q // 128
            qb = q0 // 128 + qq
            last = ci == NCH - 1
            if qq == 0 and fc == 0:
                ob_tiles[ci] = outb.tile([128, nqb * E], BF16, tag="ob",
                                         name=f"ob_{qb}")
            ob = ob_tiles[ci]
            # tail outprojs borrow the scores PSUM slots (attention is done
            # by then), keeping mm->copy->mm free of slot serialization
            tag = "scores" if tail else "proj"
            po = ps.tile([128, QC], FP32, tag=tag, bufs=2,
                         name=f"po_{qb}_{fc}")
            for mc in range(2):
                nc.tensor.matmul(
                    po[:],
                    lhsT=ctxT[mc][:, qb * 128:(qb + 1) * 128],
                    rhs=wot[mc][:, fc * QC:(fc + 1) * QC],
                    start=(mc == 0), stop=(mc == 1))
            nc.vector.tensor_copy(
                ob[:, qq * E + fc * QC: qq * E + (fc + 1) * QC], po[:])
            if last:
                # final chunk: store each fc half as soon as it's staged so
                # the exposed end-of-kernel DMA is a single small transfer
                nc.sync.dma_start(
                    out[qb * 128:(qb + 1) * 128, fc * QC:(fc + 1) * QC],
                    ob[:, qq * E + fc * QC: qq * E + (fc + 1) * QC])
                if (qq, fc) == (nqb - 1, 1):
                    del ob_tiles[ci]
            elif (qq, fc) == (nqb - 1, 1):
                nc.sync.dma_start(
                    out_3d[:, q0 // 128: q0 // 128 + nqb, :],
                    ob.rearrange("p (q f) -> p q f", f=E))
                del ob_tiles[ci]

        def outproj_pieces(ci, tail=False):
            _, Q, _ = CHUNKS[ci]
            return [lambda qq=qq, fc=fc: emit_outproj(ci, qq, fc, tail=tail)
                    for qq in range(Q // 128) for fc in range(2)]

        # ---- attention waves (one head PAIR, grp k-blocks) ----
        # kd = kb*128 - q0: offset of the k-block's diagonal within the
        # chunk's q columns. kd >= 128: cols [0, kd) are fully masked -> skip
        # in scores (exp still covers them for kd == 128; the garbage is
        # never consumed). kd >= 0: stair-mask cols [kd, kd+128).
        # For grp > 1, each wave covers grp consecutive k-blocks laid out as
        # column groups of width Q inside the head's PSUM half, sharing one
        # exp call.
        def wave_scores(ci, pair, g):
            s0, Q, grp = CHUNKS[ci]
            mt = pair
            fp8 = ci >= 1
            sc_ps = ps.tile([128, 2 * QC], FP32, tag="scores", bufs=2,
                            name=f"s_{ci}_{pair}_{g}")
            kds = [(j, (g * grp + j) * 128 - s0) for j in range(grp)]
            lo_e = 0
            for hh in range(2):
                r0 = hh * 64
                off = hh * QC
                for j, kd in kds:
                    kb = g * grp + j
                    lo = kd if (kd >= 128 and grp == 1) else 0
                    if hh == 0 and kd >= 256 and grp == 1:
                        lo_e = kd
                    if fp8:
                        # DoubleRow: dh 2x32 k-tiles, head at base 32*hh;
                        # moving free = 2*w caps piece width at 256
                        a = lo
                        while a < Q:
                            b = min(a + 256, Q)
                            nc.tensor.matmul(
                                sc_ps[:, off + j * Q + a: off + j * Q + b],
                                lhsT=k8p3[mt][32 * hh:32 * hh + 32, :,
                                              kb * 128:(kb + 1) * 128],
                                rhs=q8p3[mt][32 * hh:32 * hh + 32, :,
                                             s0 + a: s0 + b],
                                start=True, stop=True,
                                perf_mode=DRMODE)
                            a = b
                    else:
                        nc.tensor.matmul(
                            sc_ps[:, off + j * Q + lo: off + (j + 1) * Q],
                            lhsT=kTt[mt][r0:r0 + 64, kb * 128:(kb + 1) * 128],
                            rhs=qTt[mt][r0:r0 + 64, s0 + lo: s0 + Q],
                            start=True, stop=True)
            ex = expp.tile([128, 2 * QC], BF16, tag="ex",
                           name=f"e_{ci}_{pair}_{g}")
            W = grp * Q
            if lo_e or W < QC:
                # both heads in one strided-AP call: the ACT engine charges
                # by total free size, so this halves the per-call init cost
                # vs one call per head
                ex3 = ex.rearrange("p (h q) -> p h q", h=2)
                sc3 = sc_ps.rearrange("p (h q) -> p h q", h=2)
                nc.scalar.activation(ex3[:, :, lo_e:W], sc3[:, :, lo_e:W],
                                     Exp, scale=SCALE)
            else:
                nc.scalar.activation(ex[:], sc_ps[:], Exp, scale=SCALE)
            for hh in range(2):
                off = hh * QC
                for j, kd in kds:
                    if kd >= 0:
                        nc.vector.tensor_mul(
                            ex[:, off + j * Q + kd: off + j * Q + kd + 128],
                            ex[:, off + j * Q + kd: off + j * Q + kd + 128],
                            mask[:])
            return ex

        def wave_ctx(ci, pair, g, ex, ctx_pair, nkb):
            s0, Q, grp = CHUNKS[ci]
            for hh in range(2):
                h = 2 * pair + hh
                off = hh * QC
                for j in range(grp):
                    kb = g * grp + j
                    kd = kb * 128 - s0
                    lo = max(kd, 0)
                    nc.tensor.matmul(
                        ctx_pair[hh][:, lo:Q],
                        lhsT=v1_3d[:, kb * HPC + h, :],
                        rhs=ex[:, off + j * Q + lo: off + (j + 1) * Q],
                        start=(kb == 0), stop=(kb == nkb - 1),
                        skip_group_check=True)

        def emit_norm(ci, h, ctx_ps):
            s0, Q, _ = CHUNKS[ci]
            mt, r0 = h // 2, (h % 2) * 64
            # recip reads the PSUM denom row; broadcast on Pool; the norm
            # multiply reads ctx PSUM directly (no staging copy) and writes
            # bf16 ctxT. The ctx PSUM slot is released when the multiply
            # completes -- the next same-parity pair's deferred first ctx
            # matmuls give it the needed slack.
            rec = scr.tile([1, QC], FP32, tag="rec", name=f"r_{ci}_{h}")
            nc.vector.reciprocal(rec[:, 0:Q], ctx_ps[64:65, 0:Q])
            recb = scr.tile([64, QC], FP32, tag="recb", name=f"rb_{ci}_{h}")
            nc.gpsimd.partition_broadcast(recb[:, 0:Q], rec[:, 0:Q])
            nc.vector.tensor_mul(
                ctxT[mt][r0:r0 + 64, s0:s0 + Q],
                ctx_ps[0:64, 0:Q], recb[:, 0:Q])

        def emit_norm_pair(ci, items):
            # final-pair norms: interleave the two heads' recip/broadcast/
            # multiply so the DVE and Pool stages pipeline instead of
            # serializing head-by-head at the kernel tail
            s0, Q, _ = CHUNKS[ci]
            recs = []
            for h, ctx_ps in items:
                rec = scr.tile([1, QC], FP32, tag="rec", name=f"r_{ci}_{h}")
                nc.vector.reciprocal(rec[:, 0:Q], ctx_ps[64:65, 0:Q])
                recs.append(rec)
            recbs = []
            for (h, _), rec in zip(items, recs):
                recb = scr.tile([64, QC], FP32, tag="recb",
                                name=f"rb_{ci}_{h}")
                nc.gpsimd.partition_broadcast(recb[:, 0:Q], rec[:, 0:Q])
                recbs.append(recb)
            for (h, ctx_ps), recb in zip(items, recbs):
                mt, r0 = h // 2, (h % 2) * 64
                nc.vector.tensor_mul(
                    ctxT[mt][r0:r0 + 64, s0:s0 + Q],
                    ctx_ps[0:64, 0:Q], recb[:, 0:Q])

        # ---- main schedule ----
        # exp-table warm: overlap the ~2.7us table load with the initial DMA
        warm = scr.tile([1, 1], FP32, tag="warm", bufs=1, name="warm")
        nc.gpsimd.memset(warm[:], 0.0)
        nc.scalar.activation(warm[:], warm[:], Exp)

        emit_proj_qk_interleaved(0, 0)
        pending_norms = []
        for ci in range(NCH):
            q0, Q, grp = CHUNKS[ci]
            nkb = (q0 + Q) // 128
            ngrp = nkb // grp
            waves = [(pair, g) for pair in range(2)
                     for g in range(ngrp)]
            head = []   # pieces pinned to the earliest waves, one per wave
            extra = []  # pieces distributed evenly over all waves
            pins = {}   # wave -> pieces with exact placement constraints
            if ci == 0:
                qk0 = proj_qk_pieces(0)
                v0 = proj_v_pieces(range(0, 4))
                pins = {0: [qk0[2], v0[0]], 1: [qk0[3], v0[1]],
                        2: [v0[2]], 3: [v0[3]]}
                extra += proj_qk_pieces(1)[:2]
            elif ci == 1:
                head += proj_qk_pieces(1)[2:] + proj_v_pieces(range(4, 8))
                extra += proj_qk_pieces(2)[:2]
            elif ci == 2:
                head += proj_qk_pieces(2)[2:] + proj_v_pieces(range(8, 12))
                extra += proj_qk_pieces(3)[:2] + outproj_pieces(0)
            elif ci == 3:
                head += (proj_qk_pieces(3)[2:] + proj_qk_pieces(4)
                         + proj_v_pieces(range(12, 16)))
                extra += outproj_pieces(1)
            else:
                extra += outproj_pieces(2) + outproj_pieces(3)
            sched = {w: [] for w in range(len(waves))}
            for w, pcs in pins.items():
                sched[w].extend(pcs)
            for j, pc in enumerate(head):
                sched[j].append(pc)
            if extra:
                w0 = max(0, min(4, len(waves) - len(extra)))
                span_w = len(waves) - w0
                for j, pc in enumerate(extra):
                    sched[w0 + j * span_w // len(extra)].append(pc)

            ctx_tiles = {}
            ctx_queue = []
            for w, (pair, g) in enumerate(waves):
                if g == 0:
                    ctx_tiles[pair] = [
                        ps.tile([65, QC], FP32, tag="ctx", bufs=2,
                                name=f"c_{ci}_{pair}_{hh}")
                        for hh in range(2)]
                ex = wave_scores(ci, pair, g)
                if pending_norms and w in (2, 3):
                    pending_norms.pop(0)()
                for pc in sched[w]:
                    pc()
                ctx_queue.append((pair, g, ex))
                lag = max(1, 4 // grp) if g * grp < 4 else 0
                while len(ctx_queue) > lag or \
                        (ctx_queue and g == ngrp - 1):
                    qpair, qg, qex = ctx_queue.pop(0)
                    wave_ctx(ci, qpair, qg, qex, ctx_tiles[qpair], nkb)
                if g == ngrp - 1:
                    if pair == 1 and ci + 1 < NCH:
                        pending_norms = [
                            (lambda ci=ci, h=2 * pair + hh,
                                    t=ctx_tiles[pair][hh]:
                             emit_norm(ci, h, t))
                            for hh in range(2)]
                    elif pair == 1:
                        emit_norm_pair(ci, [(2 + hh, ctx_tiles[pair][hh])
                                            for hh in range(2)])
                    else:
                        emit_norm_pair(ci, [(hh, ctx_tiles[pair][hh])
                                            for hh in range(2)])
        for pc in outproj_pieces(NCH - 1, tail=True):
            pc()


def build_module():
    nc = bacc.Bacc("TRN2", target_bir_lowering=False, debug=False)
    xT = nc.dram_tensor("xT", [E, S], BF16, kind="ExternalInput").ap()
    wq = nc.dram_tensor("wq", [E, M], BF16, kind="ExternalInput").ap()
    wk = nc.dram_tensor("wk", [E, M], BF16, kind="ExternalInput").ap()
    wv = nc.dram_tensor("wv", [E, M], BF16, kind="ExternalInput").ap()
    wo = nc.dram_tensor("wo", [M, E], BF16, kind="ExternalInput").ap()
    out = nc.dram_tensor("out", [S, E], BF16, kind="ExternalOutput").ap()
    with tile.TileContext(nc) as tc:
        _emit_kernel(tc, xT, wq, wk, wv, wo, out)
    nc.compile()
    return nc


def make_in_maps(x, w_qkv):
    """Per-core input dicts (bf16, pre-transposed host-side)."""
    bf = ml_dtypes.bfloat16
    xTb = [np.ascontiguousarray(x[b].T).astype(bf) for b in range(B)]
    in_maps = []
    for c in range(NCORES):
        b, g = c // 4, c % 4
        cols = slice(g * M, (g + 1) * M)
        in_maps.append({
            "xT": xTb[b],
            "wq": np.ascontiguousarray(w_qkv[cols, :].T).astype(bf),
            "wk": np.ascontiguousarray(w_qkv[E:][cols, :].T).astype(bf),
            "wv": np.ascontiguousarray(w_qkv[2 * E:][cols, :].T).astype(bf),
            "wo": None,  # filled in kernel(), needs w_out
        })
    return in_maps


_RUNNER = None
_SHARDED = None


def _get_runner():
    """Build the Bass module once and return a cached callable
    (in_maps) -> [NCORES, S, E] bf16 partial outputs."""
    global _RUNNER
    if _RUNNER is not None:
        return _RUNNER

    nc = build_module()

    from concourse import bass2jax
    import jax
    from jax.sharding import Mesh, PartitionSpec
    from jax.experimental.shard_map import shard_map

    bass2jax.install_neuronx_cc_hook()

    in_names = ["xT", "wq", "wk", "wv", "wo"]
    out_names = ["out"]
    out_avals = [jax.core.ShapedArray((S, E), ml_dtypes.bfloat16)]
    n_params = len(in_names)
    all_names = in_names + out_names
    partition_name = (nc.partition_id_tensor.name
                      if nc.partition_id_tensor is not None else None)
    if partition_name is not None:
        all_names = all_names + [partition_name]

    def _body(*args):
        operands = list(args)
        if partition_name is not None:
            operands.append(bass2jax.partition_id_tensor())
        outs = bass2jax._bass_exec_p.bind(
            *operands,
            out_avals=tuple(out_avals),
            in_names=tuple(all_names),
            out_names=tuple(out_names),
            lowering_input_output_aliases=(),
            sim_require_finite=True,
            sim_require_nnan=True,
            nc=nc,
        )
        return tuple(outs)

    devices = jax.devices()[:NCORES]
    mesh = Mesh(np.asarray(devices), ("core",))
    n_outs = len(out_names)
    in_specs = (PartitionSpec("core"),) * (n_params + n_outs)
    out_specs = (PartitionSpec("core"),) * n_outs
    sharded = jax.jit(
        shard_map(_body, mesh=mesh, in_specs=in_specs, out_specs=out_specs,
                  check_rep=False),
        donate_argnums=tuple(range(n_params, n_params + n_outs)),
        keep_unused=True,
    )
    global _SHARDED
    _SHARDED = sharded

    def run(in_maps):
        concat_in = [
            np.concatenate([np.asarray(in_maps[c][n]) for c in range(NCORES)],
                           axis=0)
            for n in in_names
        ]
        concat_zeros = [np.zeros((NCORES * S, E), ml_dtypes.bfloat16)]
        out_arrs = sharded(*concat_in, *concat_zeros)
        return np.asarray(out_arrs[0]).reshape(NCORES, S, E)

    _RUNNER = run
    return run


def kernel(x, w_qkv, w_out, b_out):
    x = np.asarray(x, dtype=np.float32)
    w_qkv = np.asarray(w_qkv, dtype=np.float32)
    w_out = np.asarray(w_out, dtype=np.float32)
    b_out = np.asarray(b_out, dtype=np.float32)

    bf = ml_dtypes.bfloat16
    in_maps = make_in_maps(x, w_qkv)
    for c in range(NCORES):
        g = c % 4
        cols = slice(g * M, (g + 1) * M)
        in_maps[c]["wo"] = np.ascontiguousarray(w_out[:, cols].T).astype(bf)

    run = _get_runner()
    partials = run(in_maps)  # [8, S, E] bf16

    out = np.empty((B, S, E), np.float32)
    for b in range(B):
        acc = partials[4 * b].astype(np.float64)
        for i in range(1, 4):
            acc += partials[4 * b + i].astype(np.float64)
        out[b] = (acc + b_out.astype(np.float64)).astype(np.float32)
    return out


# revision 5
# speedup vs baseline: 1.2178x; 1.0058x over previous
"""Multi-head causal self-attention (B=2, S=2048, E=1024, H=16, D=64) on 8
Trainium2 NeuronCores.

Sharding: batch x head-group. Core c handles batch (c // 4) and heads
[4*(c%4), 4*(c%4)+4). Each core computes QKV projection for its 4 heads,
causal flash-attention, and a partial output projection over its head
columns. Host sums the 4 partial outputs per batch and adds b_out.

v3 changes vs the original baseline:
  - warmup dummy matmuls ride out the DMA-gated startup so the PE p-state
    ramp (0.65 -> 1.2 -> 2.4 GHz) completes before real work arrives
  - normalization reads ctx PSUM directly (no staging copy); ctx psum slot
    slack comes from the deferred-ctx lag
  - proj/outproj PSUM gets bufs=2 (removes mm->copy serialization)
  - rel=1 score blocks skip their fully-masked first 128 columns
  - partial outputs stored bf16 (halves store DMA), host sums in fp64
  - tail outproj PSUM reuses the (idle by then) scores banks
"""

import sys

if "/opt/trn_rl_repo" not in sys.path:
    sys.path.insert(0, "/opt/trn_rl_repo")

import numpy as np
import ml_dtypes

import concourse.bacc as bacc
import concourse.mybir as mybir
import concourse.tile as tile

BF16 = mybir.dt.bfloat16
FP32 = mybir.dt.float32
F8 = mybir.dt.float8e4
DRMODE = mybir.MatmulPerfMode.DoubleRow

B, S, E = 2, 2048, 1024
H, DH = 16, 64
NCORES = 8
HPC = 4            # heads per core
M = HPC * DH       # 256 ctx columns per core
QC = 512           # q chunk (max wave width; also PSUM head stride)
KB = 128           # k block
SCALE = 1.0 / np.sqrt(DH)
NWARM = 64         # warmup dummy matmuls (128 cols each)
# q-chunks (q0, Q, grp). The last 512 splits 384+128 so the final
# norm/outproj tail is 4x smaller. grp = k-blocks per wave: the narrow final
# chunk processes 4 k-blocks per scores-PSUM tile / exp call, so its waves
# are not paced by per-call ACT overhead.
CHUNKS = [(0, 512, 1), (512, 512, 1), (1024, 512, 1),
          (1536, 384, 1), (1920, 128, 4)]
NCH = len(CHUNKS)


def _emit_kernel(tc, xT, wq, wk, wv, wo_d, out):
    nc = tc.nc
    Exp = mybir.ActivationFunctionType.Exp

    with tc.tile_pool(name="res", bufs=1) as res, \
         tc.tile_pool(name="ps", bufs=1, space="PSUM") as ps, \
         tc.tile_pool(name="expp", bufs=6) as expp, \
         tc.tile_pool(name="scr", bufs=4) as scr, \
         tc.tile_pool(name="outb", bufs=2) as outb:

        # ---- resident SBUF tiles ----
        xt_all = res.tile([128, 8 * S], BF16, name="xt_all")
        xt = [xt_all[:, e * S:(e + 1) * S] for e in range(8)]
        xt_3d = xt_all.rearrange("p (e s) -> p e s", s=S)
        wqt = res.tile([128, 8 * M], BF16, name="wqt")
        wkt = res.tile([128, 8 * M], BF16, name="wkt")
        wvt = res.tile([128, 8 * M], BF16, name="wvt")
        wot = [res.tile([128, E], BF16, name=f"wot{i}") for i in range(2)]
        qTt = [res.tile([128, S], BF16, name=f"qTt{i}") for i in range(2)]
        kTt = [res.tile([128, S], BF16, name=f"kTt{i}") for i in range(2)]
        ctxT = [res.tile([128, S], BF16, name=f"ctxT{i}") for i in range(2)]
        # fp8 scores path (queries >= 512): classic-layout fp8 staging plus
        # DoubleRow "pair" tiles [64, 2*S] with head parity on partition
        # halves {0,32} and the two dh-32 k-tiles side by side in free dim
        q8c = [res.tile([128, S], F8, name=f"q8c{i}") for i in range(2)]
        k8c = [res.tile([128, S], F8, name=f"k8c{i}") for i in range(2)]
        q8p = [res.tile([64, 2 * S], F8, name=f"q8p{i}") for i in range(2)]
        k8p = [res.tile([64, 2 * S], F8, name=f"k8p{i}") for i in range(2)]
        q8p3 = [t.rearrange("p (j s) -> p j s", j=2) for t in q8p]
        k8p3 = [t.rearrange("p (j s) -> p j s", j=2) for t in k8p]
        # V with ones column: per (k-block kb, head h) a [128, 65] slab
        v1 = res.tile([128, (S // KB) * HPC * 65], BF16, name="v1")
        v1_3d = v1.rearrange("p (n c) -> p n c", c=65)
        mask = res.tile([128, 128], BF16, name="mask")
        warm_src = res.tile([128, 128], BF16, name="warm_src")

        # ---- warmup: dummy matmuls keep the PE busy (and its p-state
        # ramping) through the DMA-gated startup. They read a memset tile and
        # write a throwaway PSUM slot; the first real matmul enters a fully
        # ramped (2.4 GHz) engine.
        nc.gpsimd.memset(warm_src[:], 0.0)
        warm_ps = ps.tile([128, QC], FP32, tag="proj", bufs=2, name="warm_ps")
        for i in range(NWARM):
            nc.tensor.matmul(
                warm_ps[:, 0:128], lhsT=warm_src[:], rhs=warm_src[:],
                start=True, stop=True)

        # ---- input DMA: one batched transfer per tensor/chunk ----
        wqt_3d = wqt.rearrange("p (e m) -> p e m", m=M)
        wkt_3d = wkt.rearrange("p (e m) -> p e m", m=M)
        wvt_3d = wvt.rearrange("p (e m) -> p e m", m=M)
        xT_3d = xT.rearrange("(e p) s -> p e s", p=128)
        nc.sync.dma_start(wqt_3d[:], wq.rearrange("(e p) m -> p e m", p=128))
        nc.sync.dma_start(xt_3d[:, 0:4, 0:QC], xT_3d[:, 0:4, 0:QC])
        nc.sync.dma_start(wkt_3d[:], wk.rearrange("(e p) m -> p e m", p=128))
        nc.sync.dma_start(xt_3d[:, 4:8, 0:QC], xT_3d[:, 4:8, 0:QC])
        nc.sync.dma_start(wvt_3d[:], wv.rearrange("(e p) m -> p e m", p=128))
        for chunk in range(1, S // QC):
            nc.sync.dma_start(
                xt_3d[:, :, chunk * QC:(chunk + 1) * QC],
                xT_3d[:, :, chunk * QC:(chunk + 1) * QC])
        for i in range(2):
            nc.sync.dma_start(wot[i][:], wo_d[i * 128:(i + 1) * 128, :])

        # ---- constants ----
        nc.gpsimd.memset(v1[:], 1.0)  # data columns overwritten by V proj
        # stair mask: keep where k_local <= q_local (within a 128x128 block)
        nc.gpsimd.memset(mask[:], 1.0)
        nc.gpsimd.affine_select(
            out=mask[:], in_=mask[:],
            compare_op=mybir.AluOpType.is_ge,
            fill=0.0, base=0,
            pattern=[[1, 128]],
            channel_multiplier=-1,
        )

        # ---- emission helpers ----
        def stage_qk(ci, mt, kind, pqk):
            # chunk 0 queries score in bf16 (classic layout); all other
            # queries score in fp8 DoubleRow. k is needed in fp8 by every
            # fp8 chunk, and in bf16 only for chunk 0's k-blocks.
            s0, Q, _ = CHUNKS[ci]
            c8 = q8c if kind == "q" else k8c
            p8 = q8p3 if kind == "q" else k8p3
            dstt = qTt if kind == "q" else kTt
            if ci == 0:
                nc.vector.tensor_copy(dstt[mt][:, s0:s0 + Q], pqk[:, 0:Q])
            if kind == "k" or ci >= 1:
                nc.vector.tensor_copy(c8[mt][:, s0:s0 + Q], pqk[:, 0:Q])
                # partition rearrange into the DoubleRow pair tile:
                # pt[32*hh + d%32, (d//32)*S + s] = classic[64*hh + d, s]
                for hh in range(2):
                    for j in range(2):
                        nc.sync.dma_start(
                            p8[mt][32 * hh:32 * hh + 32, j, s0:s0 + Q],
                            c8[mt][64 * hh + 32 * j: 64 * hh + 32 * j + 32,
                                   s0:s0 + Q])

        def emit_proj_qk(ci, mt, wt, kind):
            s0, Q, _ = CHUNKS[ci]
            pqk = ps.tile([128, QC], FP32, tag="proj", bufs=2,
                          name=f"p{kind}_{ci}_{mt}")
            for e in range(8):
                nc.tensor.matmul(
                    pqk[:, 0:Q],
                    lhsT=wt[:, e * M + mt * 128: e * M + (mt + 1) * 128],
                    rhs=xt[e][:, s0:s0 + Q],
                    start=(e == 0), stop=(e == 7))
            stage_qk(ci, mt, kind, pqk)

        def emit_proj_qk_interleaved(ci, mt):
            # q and k chains interleaved per e-chunk for the DMA-gated start
            s0, Q, _ = CHUNKS[ci]
            pq = ps.tile([128, QC], FP32, tag="proj", bufs=2,
                         name=f"pqi_{ci}_{mt}")
            pk = ps.tile([128, QC], FP32, tag="proj", bufs=2,
                         name=f"pki_{ci}_{mt}")
            for e in range(8):
                nc.tensor.matmul(
                    pq[:, 0:Q],
                    lhsT=wqt[:, e * M + mt * 128: e * M + (mt + 1) * 128],
                    rhs=xt[e][:, s0:s0 + Q],
                    start=(e == 0), stop=(e == 7))
                nc.tensor.matmul(
                    pk[:, 0:Q],
                    lhsT=wkt[:, e * M + mt * 128: e * M + (mt + 1) * 128],
                    rhs=xt[e][:, s0:s0 + Q],
                    start=(e == 0), stop=(e == 7))
            stage_qk(ci, mt, "q", pq)
            stage_qk(ci, mt, "k", pk)

        def emit_proj_v(sblk):
            pv = ps.tile([128, M], FP32, tag="proj", bufs=2, name=f"pv_{sblk}")
            for e in range(8):
                nc.tensor.matmul(
                    pv[:],
                    lhsT=xt[e][:, sblk * 128:(sblk + 1) * 128],
                    rhs=wvt[:, e * M:(e + 1) * M],
                    start=(e == 0), stop=(e == 7))
            nc.vector.tensor_copy(
                v1_3d[:, sblk * HPC:(sblk + 1) * HPC, 0:64],
                pv[:].rearrange("p (h c) -> p h c", c=64))

        def proj_qk_pieces(ci):
            pcs = []
            for mt in range(2):
                pcs.append(lambda mt=mt: emit_proj_qk(ci, mt, wqt, "q"))
                pcs.append(lambda mt=mt: emit_proj_qk(ci, mt, wkt, "k"))
            return pcs

        def proj_v_pieces(blks):
            return [lambda sb=sb: emit_proj_v(sb) for sb in blks]

        ob_tiles = {}
        out_3d = out.rearrange("(q p) f -> p q f", p=128)

        def emit_outproj(ci, qq, fc, tail=False):
            q0, Q, _ = CHUNKS[ci]
            nqb = Q // 128
            qb = q0 // 128 + qq
            last = ci == NCH - 1
            if qq == 0 and fc == 0:
                ob_tiles[ci] = outb.tile([128, nqb * E], BF16, tag="ob",
                                         name=f"ob_{qb}")
            ob = ob_tiles[ci]
            # tail outprojs borrow the scores PSUM slots (attention is done
            # by then), keeping mm->copy->mm free of slot serialization
            tag = "scores" if tail else "proj"
            po = ps.tile([128, QC], FP32, tag=tag, bufs=2,
                         name=f"po_{qb}_{fc}")
            for mc in range(2):
                nc.tensor.matmul(
                    po[:],
                    lhsT=ctxT[mc][:, qb * 128:(qb + 1) * 128],
                    rhs=wot[mc][:, fc * QC:(fc + 1) * QC],
                    start=(mc == 0), stop=(mc == 1))
            if last and fc == 1:
                # final piece: stage on the (idle-by-now) ACT engine so the
                # two last copies run in parallel instead of serializing on
                # the DVE queue
                nc.scalar.activation(
                    ob[:, qq * E + fc * QC: qq * E + (fc + 1) * QC], po[:],
                    mybir.ActivationFunctionType.Copy)
            else:
                nc.vector.tensor_copy(
                    ob[:, qq * E + fc * QC: qq * E + (fc + 1) * QC], po[:])
            if last:
                # final chunk: store each fc half as soon as it's staged so
                # the exposed end-of-kernel DMA is a single small transfer
                nc.sync.dma_start(
                    out[qb * 128:(qb + 1) * 128, fc * QC:(fc + 1) * QC],
                    ob[:, qq * E + fc * QC: qq * E + (fc + 1) * QC])
                if (qq, fc) == (nqb - 1, 1):
                    del ob_tiles[ci]
            elif (qq, fc) == (nqb - 1, 1):
                nc.sync.dma_start(
                    out_3d[:, q0 // 128: q0 // 128 + nqb, :],
                    ob.rearrange("p (q f) -> p q f", f=E))
                del ob_tiles[ci]

        def outproj_pieces(ci, tail=False):
            _, Q, _ = CHUNKS[ci]
            return [lambda qq=qq, fc=fc: emit_outproj(ci, qq, fc, tail=tail)
                    for qq in range(Q // 128) for fc in range(2)]

        # ---- attention waves (one head PAIR, grp k-blocks) ----
        # kd = kb*128 - q0: offset of the k-block's diagonal within the
        # chunk's q columns. kd >= 128: cols [0, kd) are fully masked -> skip
        # in scores (exp still covers them for kd == 128; the garbage is
        # never consumed). kd >= 0: stair-mask cols [kd, kd+128).
        # For grp > 1, each wave covers grp consecutive k-blocks laid out as
        # column groups of width Q inside the head's PSUM half, sharing one
        # exp call.
        def wave_scores(ci, pair, g):
            s0, Q, grp = CHUNKS[ci]
            mt = pair
            fp8 = ci >= 1
            sc_ps = ps.tile([128, 2 * QC], FP32, tag="scores", bufs=2,
                            name=f"s_{ci}_{pair}_{g}")
            kds = [(j, (g * grp + j) * 128 - s0) for j in range(grp)]
            lo_e = 0
            for hh in range(2):
                r0 = hh * 64
                off = hh * QC
                for j, kd in kds:
                    kb = g * grp + j
                    lo = kd if (kd >= 128 and grp == 1) else 0
                    if hh == 0 and kd >= 128 and grp == 1:
                        lo_e = kd
                    if fp8:
                        # DoubleRow: dh 2x32 k-tiles, head at base 32*hh;
                        # moving free = 2*w caps piece width at 256
                        a = lo
                        while a < Q:
                            b = min(a + 256, Q)
                            nc.tensor.matmul(
                                sc_ps[:, off + j * Q + a: off + j * Q + b],
                                lhsT=k8p3[mt][32 * hh:32 * hh + 32, :,
                                              kb * 128:(kb + 1) * 128],
                                rhs=q8p3[mt][32 * hh:32 * hh + 32, :,
                                             s0 + a: s0 + b],
                                start=True, stop=True,
                                perf_mode=DRMODE)
                            a = b
                    else:
                        nc.tensor.matmul(
                            sc_ps[:, off + j * Q + lo: off + (j + 1) * Q],
                            lhsT=kTt[mt][r0:r0 + 64, kb * 128:(kb + 1) * 128],
                            rhs=qTt[mt][r0:r0 + 64, s0 + lo: s0 + Q],
                            start=True, stop=True)
            ex = expp.tile([128, 2 * QC], BF16, tag="ex",
                           name=f"e_{ci}_{pair}_{g}")
            W = grp * Q
            if lo_e or W < QC:
                # both heads in one strided-AP call: the ACT engine charges
                # by total free size, so this halves the per-call init cost
                # vs one call per head
                ex3 = ex.rearrange("p (h q) -> p h q", h=2)
                sc3 = sc_ps.rearrange("p (h q) -> p h q", h=2)
                nc.scalar.activation(ex3[:, :, lo_e:W], sc3[:, :, lo_e:W],
                                     Exp, scale=SCALE)
            else:
                nc.scalar.activation(ex[:], sc_ps[:], Exp, scale=SCALE)
            for hh in range(2):
                off = hh * QC
                for j, kd in kds:
                    if kd >= 0:
                        nc.vector.tensor_mul(
                            ex[:, off + j * Q + kd: off + j * Q + kd + 128],
                            ex[:, off + j * Q + kd: off + j * Q + kd + 128],
                            mask[:])
            return ex

        def wave_ctx(ci, pair, g, ex, ctx_pair, nkb):
            s0, Q, grp = CHUNKS[ci]
            for hh in range(2):
                h = 2 * pair + hh
                off = hh * QC
                for j in range(grp):
                    kb = g * grp + j
                    kd = kb * 128 - s0
                    lo = max(kd, 0)
                    nc.tensor.matmul(
                        ctx_pair[hh][:, lo:Q],
                        lhsT=v1_3d[:, kb * HPC + h, :],
                        rhs=ex[:, off + j * Q + lo: off + (j + 1) * Q],
                        start=(kb == 0), stop=(kb == nkb - 1),
                        skip_group_check=True)

        def norm_pieces(ci, items):
            # deferred norm for a finished pair, split into 4 wave-pieces
            # (recips / broadcasts / mul A / mul B) so the multiplies never
            # head-of-line-block the in-order DVE queue waiting on the Pool
            # broadcasts. The norm multiply reads ctx PSUM directly (no
            # staging copy); the slot is released when it completes.
            s0, Q, _ = CHUNKS[ci]
            state = {}

            def p_recips():
                state["recs"] = []
                for h, ctx_ps in items:
                    rec = scr.tile([1, QC], FP32, tag="rec",
                                   name=f"r_{ci}_{h}")
                    nc.vector.reciprocal(rec[:, 0:Q], ctx_ps[64:65, 0:Q])
                    state["recs"].append(rec)

            def p_bcasts():
                state["recbs"] = []
                for (h, _), rec in zip(items, state["recs"]):
                    recb = scr.tile([64, QC], FP32, tag="recb",
                                    name=f"rb_{ci}_{h}")
                    nc.gpsimd.partition_broadcast(recb[:, 0:Q], rec[:, 0:Q])
                    state["recbs"].append(recb)

            def p_mul(i):
                h, ctx_ps = items[i]
                mt, r0 = h // 2, (h % 2) * 64
                nc.vector.tensor_mul(
                    ctxT[mt][r0:r0 + 64, s0:s0 + Q],
                    ctx_ps[0:64, 0:Q], state["recbs"][i][:, 0:Q])

            return [p_recips, p_bcasts,
                    lambda: p_mul(0), lambda: p_mul(1)]

        def emit_norm_pair(ci, items):
            # final-pair norms: interleave the two heads' recip/broadcast/
            # multiply so the DVE and Pool stages pipeline instead of
            # serializing head-by-head at the kernel tail
            s0, Q, _ = CHUNKS[ci]
            recs = []
            for h, ctx_ps in items:
                rec = scr.tile([1, QC], FP32, tag="rec", name=f"r_{ci}_{h}")
                nc.vector.reciprocal(rec[:, 0:Q], ctx_ps[64:65, 0:Q])
                recs.append(rec)
            recbs = []
            for (h, _), rec in zip(items, recs):
                recb = scr.tile([64, QC], FP32, tag="recb",
                                name=f"rb_{ci}_{h}")
                nc.gpsimd.partition_broadcast(recb[:, 0:Q], rec[:, 0:Q])
                recbs.append(recb)
            for (h, ctx_ps), recb in zip(items, recbs):
                mt, r0 = h // 2, (h % 2) * 64
                nc.vector.tensor_mul(
                    ctxT[mt][r0:r0 + 64, s0:s0 + Q],
                    ctx_ps[0:64, 0:Q], recb[:, 0:Q])

        # ---- main schedule ----
        # exp-table warm: overlap the ~2.7us table load with the initial DMA
        warm = scr.tile([1, 1], FP32, tag="warm", bufs=1, name="warm")
        nc.gpsimd.memset(warm[:], 0.0)
        nc.scalar.activation(warm[:], warm[:], Exp)

        emit_proj_qk_interleaved(0, 0)
        pending_norms = []
        for ci in range(NCH):
            q0, Q, grp = CHUNKS[ci]
            nkb = (q0 + Q) // 128
            ngrp = nkb // grp
            waves = [(pair, g) for pair in range(2)
                     for g in range(ngrp)]
            head = []   # pieces pinned to the earliest waves, one per wave
            extra = []  # pieces distributed evenly over all waves
            pins = {}   # wave -> pieces with exact placement constraints
            if ci == 0:
                qk0 = proj_qk_pieces(0)
                v0 = proj_v_pieces(range(0, 4))
                pins = {0: [qk0[2], v0[0]], 1: [qk0[3], v0[1]],
                        2: [v0[2]], 3: [v0[3]]}
                extra += proj_qk_pieces(1)[:2]
            elif ci == 1:
                head += proj_qk_pieces(1)[2:] + proj_v_pieces(range(4, 8))
                extra += proj_qk_pieces(2)[:2]
            elif ci == 2:
                head += proj_qk_pieces(2)[2:] + proj_v_pieces(range(8, 12))
                extra += proj_qk_pieces(3)[:2] + outproj_pieces(0)
            elif ci == 3:
                head += (proj_qk_pieces(3)[2:] + proj_qk_pieces(4)
                         + proj_v_pieces(range(12, 16)))
                extra += outproj_pieces(1)
            else:
                extra += outproj_pieces(2) + outproj_pieces(3)
            sched = {w: [] for w in range(len(waves))}
            for w, pcs in pins.items():
                sched[w].extend(pcs)
            for j, pc in enumerate(head):
                sched[j].append(pc)
            if extra:
                w0 = max(0, min(4, len(waves) - len(extra)))
                span_w = len(waves) - w0
                for j, pc in enumerate(extra):
                    sched[w0 + j * span_w // len(extra)].append(pc)

            ctx_tiles = {}
            ctx_queue = []
            for w, (pair, g) in enumerate(waves):
                if g == 0:
                    ctx_tiles[pair] = [
                        ps.tile([65, QC], FP32, tag="ctx", bufs=2,
                                name=f"c_{ci}_{pair}_{hh}")
                        for hh in range(2)]
                ex = wave_scores(ci, pair, g)
                if pending_norms:
                    pending_norms.pop(0)()
                last_of_pair = g == ngrp - 1
                final_pair = last_of_pair and pair == 1 and ci + 1 == NCH
                if not final_pair:
                    for pc in sched[w]:
                        pc()
                ctx_queue.append((pair, g, ex))
                # defer ctx so (a) the pair's first ctx matmuls don't stall
                # on PSUM slots still being normed, (b) PE has scores to run
                # while exp catches up. Grouped chunks defer until wave 3 so
                # the previous pair's lazily-spread norm muls (waves 2,3)
                # have released the slots.
                if grp > 1:
                    lag = max(0, 3 - g)
                else:
                    lag = 4 if g < 4 else 0
                while len(ctx_queue) > lag or \
                        (ctx_queue and last_of_pair):
                    qpair, qg, qex = ctx_queue.pop(0)
                    wave_ctx(ci, qpair, qg, qex, ctx_tiles[qpair], nkb)
                if last_of_pair:
                    h0 = 2 * pair
                    items = [(h0 + hh, ctx_tiles[pair][hh])
                             for hh in range(2)]
                    if final_pair:
                        # kernel tail: emit the norm chain ahead of this
                        # wave's filler copies so the recips don't queue
                        # behind them on the in-order DVE
                        emit_norm_pair(ci, items)
                        for pc in sched[w]:
                            pc()
                    else:
                        # lazily spread over the next 4 waves (pair 0's run
                        # inside this chunk's pair-1 waves; pair 1's inside
                        # the next chunk)
                        pending_norms = norm_pieces(ci, items)
        for pc in outproj_pieces(NCH - 1, tail=True):
            pc()


def build_module():
    nc = bacc.Bacc("TRN2", target_bir_lowering=False, debug=False)
    xT = nc.dram_tensor("xT", [E, S], BF16, kind="ExternalInput").ap()
    wq = nc.dram_tensor("wq", [E, M], BF16, kind="ExternalInput").ap()
    wk = nc.dram_tensor("wk", [E, M], BF16, kind="ExternalInput").ap()
    wv = nc.dram_tensor("wv", [E, M], BF16, kind="ExternalInput").ap()
    wo = nc.dram_tensor("wo", [M, E], BF16, kind="ExternalInput").ap()
    out = nc.dram_tensor("out", [S, E], BF16, kind="ExternalOutput").ap()
    with tile.TileContext(nc) as tc:
        _emit_kernel(tc, xT, wq, wk, wv, wo, out)
    nc.compile()
    return nc


def make_in_maps(x, w_qkv):
    """Per-core input dicts (bf16, pre-transposed host-side)."""
    bf = ml_dtypes.bfloat16
    xTb = [np.ascontiguousarray(x[b].T).astype(bf) for b in range(B)]
    in_maps = []
    for c in range(NCORES):
        b, g = c // 4, c % 4
        cols = slice(g * M, (g + 1) * M)
        in_maps.append({
            "xT": xTb[b],
            "wq": np.ascontiguousarray(w_qkv[cols, :].T).astype(bf),
            "wk": np.ascontiguousarray(w_qkv[E:][cols, :].T).astype(bf),
            "wv": np.ascontiguousarray(w_qkv[2 * E:][cols, :].T).astype(bf),
            "wo": None,  # filled in kernel(), needs w_out
        })
    return in_maps


_RUNNER = None
_SHARDED = None


def _get_runner():
    """Build the Bass module once and return a cached callable
    (in_maps) -> [NCORES, S, E] bf16 partial outputs."""
    global _RUNNER
    if _RUNNER is not None:
        return _RUNNER

    nc = build_module()

    from concourse import bass2jax
    import jax
    from jax.sharding import Mesh, PartitionSpec
    from jax.experimental.shard_map import shard_map

    bass2jax.install_neuronx_cc_hook()

    in_names = ["xT", "wq", "wk", "wv", "wo"]
    out_names = ["out"]
    out_avals = [jax.core.ShapedArray((S, E), ml_dtypes.bfloat16)]
    n_params = len(in_names)
    all_names = in_names + out_names
    partition_name = (nc.partition_id_tensor.name
                      if nc.partition_id_tensor is not None else None)
    if partition_name is not None:
        all_names = all_names + [partition_name]

    def _body(*args):
        operands = list(args)
        if partition_name is not None:
            operands.append(bass2jax.partition_id_tensor())
        outs = bass2jax._bass_exec_p.bind(
            *operands,
            out_avals=tuple(out_avals),
            in_names=tuple(all_names),
            out_names=tuple(out_names),
            lowering_input_output_aliases=(),
            sim_require_finite=True,
            sim_require_nnan=True,
            nc=nc,
        )
        return tuple(outs)

    devices = jax.devices()[:NCORES]
    mesh = Mesh(np.asarray(devices), ("core",))
    n_outs = len(out_names)
    in_specs = (PartitionSpec("core"),) * (n_params + n_outs)
    out_specs = (PartitionSpec("core"),) * n_outs
    sharded = jax.jit(
        shard_map(_body, mesh=mesh, in_specs=in_specs, out_specs=out_specs,
                  check_rep=False),
        donate_argnums=tuple(range(n_params, n_params + n_outs)),
        keep_unused=True,
    )
    global _SHARDED
    _SHARDED = sharded

    def run(in_maps):
        concat_in = [
            np.concatenate([np.asarray(in_maps[c][n]) for c in range(NCORES)],
                           axis=0)
            for n in in_names
        ]
        concat_zeros = [np.zeros((NCORES * S, E), ml_dtypes.bfloat16)]
        out_arrs = sharded(*concat_in, *concat_zeros)
        return np.asarray(out_arrs[0]).reshape(NCORES, S, E)

    _RUNNER = run
    return run


def kernel(x, w_qkv, w_out, b_out):
    x = np.asarray(x, dtype=np.float32)
    w_qkv = np.asarray(w_qkv, dtype=np.float32)
    w_out = np.asarray(w_out, dtype=np.float32)
    b_out = np.asarray(b_out, dtype=np.float32)

    bf = ml_dtypes.bfloat16
    in_maps = make_in_maps(x, w_qkv)
    for c in range(NCORES):
        g = c % 4
        cols = slice(g * M, (g + 1) * M)
        in_maps[c]["wo"] = np.ascontiguousarray(w_out[:, cols].T).astype(bf)

    run = _get_runner()
    partials = run(in_maps)  # [8, S, E] bf16

    out = np.empty((B, S, E), np.float32)
    for b in range(B):
        acc = partials[4 * b].astype(np.float64)
        for i in range(1, 4):
            acc += partials[4 * b + i].astype(np.float64)
        out[b] = (acc + b_out.astype(np.float64)).astype(np.float32)
    return out


# revision 8
# speedup vs baseline: 1.2849x; 1.0551x over previous
"""Multi-head causal self-attention (B=2, S=2048, E=1024, H=16, D=64) on 8
Trainium2 NeuronCores.

Sharding: batch x head-group. Core c handles batch (c // 4) and heads
[4*(c%4), 4*(c%4)+4). Each core computes QKV projection for its 4 heads,
causal flash-attention, and a partial output projection over its head
columns. Host sums the 4 partial outputs per batch and adds b_out.

v3 changes vs the original baseline:
  - warmup dummy matmuls ride out the DMA-gated startup so the PE p-state
    ramp (0.65 -> 1.2 -> 2.4 GHz) completes before real work arrives
  - normalization reads ctx PSUM directly (no staging copy); ctx psum slot
    slack comes from the deferred-ctx lag
  - proj/outproj PSUM gets bufs=2 (removes mm->copy serialization)
  - rel=1 score blocks skip their fully-masked first 128 columns
  - partial outputs stored bf16 (halves store DMA), host sums in fp64
  - tail outproj PSUM reuses the (idle by then) scores banks
"""

import sys

if "/opt/trn_rl_repo" not in sys.path:
    sys.path.insert(0, "/opt/trn_rl_repo")

import numpy as np
import ml_dtypes

import concourse.bacc as bacc
import concourse.mybir as mybir
import concourse.tile as tile

BF16 = mybir.dt.bfloat16
FP32 = mybir.dt.float32
F8 = mybir.dt.float8e4
DRMODE = mybir.MatmulPerfMode.DoubleRow

B, S, E = 2, 2048, 1024
H, DH = 16, 64
NCORES = 8
HPC = 4            # heads per core
M = HPC * DH       # 256 ctx columns per core
QC = 512           # q chunk (max wave width; also PSUM head stride)
KB = 128           # k block
SCALE = 1.0 / np.sqrt(DH)
NWARM = 64         # warmup dummy matmuls (128 cols each)
# q-chunks (q0, Q, grp). The last 512 splits 384+128 so the final
# norm/outproj tail is 4x smaller. grp = k-blocks per wave: the narrow final
# chunk processes 4 k-blocks per scores-PSUM tile / exp call, so its waves
# are not paced by per-call ACT overhead.
CHUNKS = [(0, 512, 1), (512, 512, 1), (1024, 512, 1),
          (1536, 384, 1), (1920, 128, 4)]
NCH = len(CHUNKS)


def _emit_kernel(tc, xT, wq, wk, wv, wo_d, x8, wk8, out):
    nc = tc.nc
    Exp = mybir.ActivationFunctionType.Exp

    with tc.tile_pool(name="res", bufs=1) as res, \
         tc.tile_pool(name="ps", bufs=1, space="PSUM") as ps, \
         tc.tile_pool(name="expp", bufs=6) as expp, \
         tc.tile_pool(name="scr", bufs=4) as scr, \
         tc.tile_pool(name="outb", bufs=2) as outb:

        # ---- resident SBUF tiles ----
        xt_all = res.tile([128, 8 * S], BF16, name="xt_all")
        xt = [xt_all[:, e * S:(e + 1) * S] for e in range(8)]
        xt_3d = xt_all.rearrange("p (e s) -> p e s", s=S)
        wqt = res.tile([128, 8 * M], BF16, name="wqt")
        wkt = res.tile([128, 8 * M], BF16, name="wkt")
        wvt = res.tile([128, 8 * M], BF16, name="wvt")
        wot = [res.tile([128, E], BF16, name=f"wot{i}") for i in range(2)]
        qTt = [res.tile([128, S], BF16, name=f"qTt{i}") for i in range(2)]
        kTt = [res.tile([128, S], BF16, name=f"kTt{i}") for i in range(2)]
        ctxT = [res.tile([128, S], BF16, name=f"ctxT{i}") for i in range(2)]
        # fp8 scores path (queries >= 512): one classic-layout fp8 staging
        # tile (free dims qk x mt x s) and one DoubleRow "pair" tile
        # [64, j x qk x mt x s] with head parity on partition halves {0,32}
        # and the two dh-32 k-tiles (j) in the free dim. Folding qk/mt into
        # free dims lets ONE rearrange DMA per (hh, j) re-stage a whole
        # chunk (HWDGE generation is a fixed 625ns per dma_start).
        qk8c = res.tile([128, 4 * S], F8, name="qk8c")
        qk8c4 = qk8c.rearrange("p (t m s) -> p t m s", t=2, m=2)
        qk8p = res.tile([64, 8 * S], F8, name="qk8p")
        qk8p5 = qk8p.rearrange("p (j t m s) -> p j t m s", j=2, t=2, m=2)
        # fp8 k-projection operands (host-packed e-pair layout): the
        # k-columns >= 512 (only ever consumed by the fp8 score path) are
        # projected with fp8 DoubleRow matmuls at 1/4 the PE cost
        x8t = res.tile([128, 8 * S], F8, name="x8t")
        x8_4d = x8t.rearrange("p (j t s) -> p j t s", j=4, t=2)
        wk8t = res.tile([128, 8 * M], F8, name="wk8t")
        wk8_4d = wk8t.rearrange("p (j t m) -> p j t m", j=4, t=2)
        # V with ones column: per (k-block kb, head h) a [128, 65] slab
        v1 = res.tile([128, (S // KB) * HPC * 65], BF16, name="v1")
        v1_3d = v1.rearrange("p (n c) -> p n c", c=65)
        mask = res.tile([128, 128], BF16, name="mask")
        warm_src = res.tile([128, 128], BF16, name="warm_src")

        # ---- warmup: dummy matmuls keep the PE busy (and its p-state
        # ramping) through the DMA-gated startup. They read a memset tile and
        # write a throwaway PSUM slot; the first real matmul enters a fully
        # ramped (2.4 GHz) engine.
        nc.gpsimd.memset(warm_src[:], 0.0)
        warm_ps = ps.tile([128, QC], FP32, tag="proj", bufs=2, name="warm_ps")
        for i in range(NWARM):
            nc.tensor.matmul(
                warm_ps[:, 0:128], lhsT=warm_src[:], rhs=warm_src[:],
                start=True, stop=True)

        # ---- input DMA: one batched transfer per tensor/chunk ----
        wqt_3d = wqt.rearrange("p (e m) -> p e m", m=M)
        wkt_3d = wkt.rearrange("p (e m) -> p e m", m=M)
        wvt_3d = wvt.rearrange("p (e m) -> p e m", m=M)
        xT_3d = xT.rearrange("(e p) s -> p e s", p=128)
        nc.sync.dma_start(wqt_3d[:], wq.rearrange("(e p) m -> p e m", p=128))
        nc.sync.dma_start(xt_3d[:, 0:4, 0:QC], xT_3d[:, 0:4, 0:QC])
        nc.sync.dma_start(wkt_3d[:], wk.rearrange("(e p) m -> p e m", p=128))
        nc.sync.dma_start(xt_3d[:, 4:8, 0:QC], xT_3d[:, 4:8, 0:QC])
        nc.sync.dma_start(wvt_3d[:], wv.rearrange("(e p) m -> p e m", p=128))
        x8_dram = x8.rearrange("p (j t s) -> p j t s", j=4, t=2)
        nc.sync.dma_start(wk8t[:], wk8)
        for chunk in range(1, S // QC):
            nc.sync.dma_start(
                x8_4d[:, :, :, chunk * QC:(chunk + 1) * QC],
                x8_dram[:, :, :, chunk * QC:(chunk + 1) * QC])
            nc.sync.dma_start(
                xt_3d[:, :, chunk * QC:(chunk + 1) * QC],
                xT_3d[:, :, chunk * QC:(chunk + 1) * QC])
        for i in range(2):
            nc.sync.dma_start(wot[i][:], wo_d[i * 128:(i + 1) * 128, :])

        # ---- constants ----
        nc.gpsimd.memset(v1[:], 1.0)  # data columns overwritten by V proj
        # stair mask: keep where k_local <= q_local (within a 128x128 block)
        nc.gpsimd.memset(mask[:], 1.0)
        nc.gpsimd.affine_select(
            out=mask[:], in_=mask[:],
            compare_op=mybir.AluOpType.is_ge,
            fill=0.0, base=0,
            pattern=[[1, 128]],
            channel_multiplier=-1,
        )

        # ---- emission helpers ----
        def stage_f8(ci, mt, kind, pqk, scale=None):
            # fp8 classic staging into the (qk, mt) slab of qk8c; the
            # partition rearrange into qk8p is a separate batched piece
            s0, Q, _ = CHUNKS[ci]
            t = 0 if kind == "q" else 1
            if scale is None:
                nc.vector.tensor_copy(qk8c4[:, t, mt, s0:s0 + Q], pqk[:, 0:Q])
            else:
                nc.vector.tensor_scalar_mul(qk8c4[:, t, mt, s0:s0 + Q],
                                            pqk[:, 0:Q], scale)

        def emit_rearrange(ci, k_only=False):
            # partition rearrange into the DoubleRow pair tile for chunk
            # ci's columns, all (qk, mt) slabs at once:
            # pt[32*hh + d%32, (d//32), t, m, s] = classic[64*hh + d, t, m, s]
            s0, Q, _ = CHUNKS[ci]
            t0 = 1 if k_only else 0
            for hh in range(2):
                for j in range(2):
                    nc.sync.dma_start(
                        qk8p5[32 * hh:32 * hh + 32, j, t0:2, :, s0:s0 + Q],
                        qk8c4[64 * hh + 32 * j: 64 * hh + 32 * j + 32,
                              t0:2, :, s0:s0 + Q])

        def stage_qk(ci, mt, kind, pqk):
            # chunk 0 queries score in bf16 (classic layout); all other
            # queries score in fp8 DoubleRow. k is needed in fp8 by every
            # fp8 chunk, and in bf16 only for chunk 0's k-blocks.
            s0, Q, _ = CHUNKS[ci]
            dstt = qTt if kind == "q" else kTt
            if ci == 0:
                nc.vector.tensor_copy(dstt[mt][:, s0:s0 + Q], pqk[:, 0:Q])
            if kind == "k" or ci >= 1:
                stage_f8(ci, mt, kind, pqk)

        def emit_proj_k8(ci, mt):
            # k-projection for fp8-only consumers via fp8 DoubleRow over
            # host-packed e-pairs: 1/4 the PE cost of the bf16 projection
            s0, Q, _ = CHUNKS[ci]
            pk = ps.tile([128, QC], FP32, tag="proj", bufs=2,
                         name=f"pk8_{ci}_{mt}")
            # a-piece OUTER: interleaving two DoubleRow accumulation groups
            # (j inner per region) miscomputes on hardware -- each region's
            # 4-instruction group must run contiguously
            for a in range(0, Q, 256):
                b = min(a + 256, Q)
                for j in range(4):
                    nc.tensor.matmul(
                        pk[:, a:b],
                        lhsT=wk8_4d[:, j, :, mt * 128:(mt + 1) * 128],
                        rhs=x8_4d[:, j, :, s0 + a: s0 + b],
                        start=(j == 0), stop=(j == 3),
                        perf_mode=DRMODE)
            # wk8 is host-scaled by 64 (w values ~0.02 sit in e4m3's
            # subnormal range, which the PE flushes to zero); undo here
            stage_f8(ci, mt, "k", pk, scale=1.0 / 64.0)

        def emit_proj_qk(ci, mt, wt, kind):
            s0, Q, _ = CHUNKS[ci]
            pqk = ps.tile([128, QC], FP32, tag="proj", bufs=2,
                          name=f"p{kind}_{ci}_{mt}")
            for e in range(8):
                nc.tensor.matmul(
                    pqk[:, 0:Q],
                    lhsT=wt[:, e * M + mt * 128: e * M + (mt + 1) * 128],
                    rhs=xt[e][:, s0:s0 + Q],
                    start=(e == 0), stop=(e == 7))
            stage_qk(ci, mt, kind, pqk)

        def emit_proj_qk_interleaved(ci, mt):
            # q and k chains interleaved per e-chunk for the DMA-gated start
            s0, Q, _ = CHUNKS[ci]
            pq = ps.tile([128, QC], FP32, tag="proj", bufs=2,
                         name=f"pqi_{ci}_{mt}")
            pk = ps.tile([128, QC], FP32, tag="proj", bufs=2,
                         name=f"pki_{ci}_{mt}")
            for e in range(8):
                nc.tensor.matmul(
                    pq[:, 0:Q],
                    lhsT=wqt[:, e * M + mt * 128: e * M + (mt + 1) * 128],
                    rhs=xt[e][:, s0:s0 + Q],
                    start=(e == 0), stop=(e == 7))
                nc.tensor.matmul(
                    pk[:, 0:Q],
                    lhsT=wkt[:, e * M + mt * 128: e * M + (mt + 1) * 128],
                    rhs=xt[e][:, s0:s0 + Q],
                    start=(e == 0), stop=(e == 7))
            stage_qk(ci, mt, "q", pq)
            stage_qk(ci, mt, "k", pk)

        def emit_proj_v(sblk):
            pv = ps.tile([128, M], FP32, tag="proj", bufs=2, name=f"pv_{sblk}")
            for e in range(8):
                nc.tensor.matmul(
                    pv[:],
                    lhsT=xt[e][:, sblk * 128:(sblk + 1) * 128],
                    rhs=wvt[:, e * M:(e + 1) * M],
                    start=(e == 0), stop=(e == 7))
            nc.vector.tensor_copy(
                v1_3d[:, sblk * HPC:(sblk + 1) * HPC, 0:64],
                pv[:].rearrange("p (h c) -> p h c", c=64))

        def proj_qk_pieces(ci):
            pcs = []
            for mt in range(2):
                pcs.append(lambda mt=mt: emit_proj_qk(ci, mt, wqt, "q"))
                if ci == 0:
                    pcs.append(lambda mt=mt: emit_proj_qk(ci, mt, wkt, "k"))
                else:
                    pcs.append(lambda mt=mt: emit_proj_k8(ci, mt))
            return pcs

        def proj_v_pieces(blks):
            return [lambda sb=sb: emit_proj_v(sb) for sb in blks]

        ob_tiles = {}
        out_3d = out.rearrange("(q p) f -> p q f", p=128)

        def emit_outproj(ci, qq, fc, tail=False):
            q0, Q, _ = CHUNKS[ci]
            nqb = Q // 128
            qb = q0 // 128 + qq
            last = ci == NCH - 1
            if qq == 0 and fc == 0:
                ob_tiles[ci] = outb.tile([128, nqb * E], BF16, tag="ob",
                                         name=f"ob_{qb}")
            ob = ob_tiles[ci]
            # tail outprojs borrow the scores PSUM slots (attention is done
            # by then), keeping mm->copy->mm free of slot serialization
            tag = "scores" if tail else "proj"
            po = ps.tile([128, QC], FP32, tag=tag, bufs=2,
                         name=f"po_{qb}_{fc}")
            for mc in range(2):
                nc.tensor.matmul(
                    po[:],
                    lhsT=ctxT[mc][:, qb * 128:(qb + 1) * 128],
                    rhs=wot[mc][:, fc * QC:(fc + 1) * QC],
                    start=(mc == 0), stop=(mc == 1))
            if last and fc == 1:
                # final piece: stage on the (idle-by-now) ACT engine so the
                # two last copies run in parallel instead of serializing on
                # the DVE queue
                nc.scalar.activation(
                    ob[:, qq * E + fc * QC: qq * E + (fc + 1) * QC], po[:],
                    mybir.ActivationFunctionType.Copy)
            else:
                nc.vector.tensor_copy(
                    ob[:, qq * E + fc * QC: qq * E + (fc + 1) * QC], po[:])
            if last:
                # final chunk: store each fc half as soon as it's staged so
                # the exposed end-of-kernel DMA is a single small transfer
                nc.sync.dma_start(
                    out[qb * 128:(qb + 1) * 128, fc * QC:(fc + 1) * QC],
                    ob[:, qq * E + fc * QC: qq * E + (fc + 1) * QC])
                if (qq, fc) == (nqb - 1, 1):
                    del ob_tiles[ci]
            elif (qq, fc) == (nqb - 1, 1):
                nc.sync.dma_start(
                    out_3d[:, q0 // 128: q0 // 128 + nqb, :],
                    ob.rearrange("p (q f) -> p q f", f=E))
                del ob_tiles[ci]

        def outproj_pieces(ci, tail=False):
            _, Q, _ = CHUNKS[ci]
            return [lambda qq=qq, fc=fc: emit_outproj(ci, qq, fc, tail=tail)
                    for qq in range(Q // 128) for fc in range(2)]

        # ---- attention waves (one head PAIR, grp k-blocks) ----
        # kd = kb*128 - q0: offset of the k-block's diagonal within the
        # chunk's q columns. kd >= 128: cols [0, kd) are fully masked -> skip
        # in scores (exp still covers them for kd == 128; the garbage is
        # never consumed). kd >= 0: stair-mask cols [kd, kd+128).
        # For grp > 1, each wave covers grp consecutive k-blocks laid out as
        # column groups of width Q inside the head's PSUM half, sharing one
        # exp call.
        def wave_scores(ci, pair, g):
            s0, Q, grp = CHUNKS[ci]
            mt = pair
            fp8 = ci >= 1
            sc_ps = ps.tile([128, 2 * QC], FP32, tag="scores", bufs=2,
                            name=f"s_{ci}_{pair}_{g}")
            kds = [(j, (g * grp + j) * 128 - s0) for j in range(grp)]
            lo_e = 0
            for hh in range(2):
                r0 = hh * 64
                off = hh * QC
                for j, kd in kds:
                    kb = g * grp + j
                    lo = kd if (kd >= 128 and grp == 1) else 0
                    if hh == 0 and kd >= 128 and grp == 1:
                        lo_e = kd
                    if fp8:
                        # DoubleRow: dh 2x32 k-tiles, head at base 32*hh;
                        # moving free = 2*w caps piece width at 256
                        a = lo
                        while a < Q:
                            b = min(a + 256, Q)
                            nc.tensor.matmul(
                                sc_ps[:, off + j * Q + a: off + j * Q + b],
                                lhsT=qk8p5[32 * hh:32 * hh + 32, :, 1, mt,
                                           kb * 128:(kb + 1) * 128],
                                rhs=qk8p5[32 * hh:32 * hh + 32, :, 0, mt,
                                          s0 + a: s0 + b],
                                start=True, stop=True,
                                perf_mode=DRMODE)
                            a = b
                    else:
                        nc.tensor.matmul(
                            sc_ps[:, off + j * Q + lo: off + (j + 1) * Q],
                            lhsT=kTt[mt][r0:r0 + 64, kb * 128:(kb + 1) * 128],
                            rhs=qTt[mt][r0:r0 + 64, s0 + lo: s0 + Q],
                            start=True, stop=True)
            ex = expp.tile([128, 2 * QC], BF16, tag="ex",
                           name=f"e_{ci}_{pair}_{g}")
            W = grp * Q
            if lo_e or W < QC:
                # both heads in one strided-AP call: the ACT engine charges
                # by total free size, so this halves the per-call init cost
                # vs one call per head
                ex3 = ex.rearrange("p (h q) -> p h q", h=2)
                sc3 = sc_ps.rearrange("p (h q) -> p h q", h=2)
                nc.scalar.activation(ex3[:, :, lo_e:W], sc3[:, :, lo_e:W],
                                     Exp, scale=SCALE)
            else:
                nc.scalar.activation(ex[:], sc_ps[:], Exp, scale=SCALE)
            for hh in range(2):
                off = hh * QC
                for j, kd in kds:
                    if kd >= 0:
                        nc.vector.tensor_mul(
                            ex[:, off + j * Q + kd: off + j * Q + kd + 128],
                            ex[:, off + j * Q + kd: off + j * Q + kd + 128],
                            mask[:])
            return ex

        def wave_ctx(ci, pair, g, ex, ctx_pair, nkb):
            s0, Q, grp = CHUNKS[ci]
            for hh in range(2):
                h = 2 * pair + hh
                off = hh * QC
                for j in range(grp):
                    kb = g * grp + j
                    kd = kb * 128 - s0
                    lo = max(kd, 0)
                    nc.tensor.matmul(
                        ctx_pair[hh][:, lo:Q],
                        lhsT=v1_3d[:, kb * HPC + h, :],
                        rhs=ex[:, off + j * Q + lo: off + (j + 1) * Q],
                        start=(kb == 0), stop=(kb == nkb - 1),
                        skip_group_check=True)

        def norm_pieces(ci, items):
            # deferred norm for a finished pair, split into 4 wave-pieces
            # (recips / broadcasts / mul A / mul B) so the multiplies never
            # head-of-line-block the in-order DVE queue waiting on the Pool
            # broadcasts. The norm multiply reads ctx PSUM directly (no
            # staging copy); the slot is released when it completes.
            s0, Q, _ = CHUNKS[ci]
            state = {}

            def p_recips():
                state["recs"] = []
                for h, ctx_ps in items:
                    rec = scr.tile([1, QC], FP32, tag="rec",
                                   name=f"r_{ci}_{h}")
                    nc.vector.reciprocal(rec[:, 0:Q], ctx_ps[64:65, 0:Q])
                    state["recs"].append(rec)

            def p_bcasts():
                state["recbs"] = []
                for (h, _), rec in zip(items, state["recs"]):
                    recb = scr.tile([64, QC], FP32, tag="recb",
                                    name=f"rb_{ci}_{h}")
                    nc.gpsimd.partition_broadcast(recb[:, 0:Q], rec[:, 0:Q])
                    state["recbs"].append(recb)

            def p_mul(i):
                h, ctx_ps = items[i]
                mt, r0 = h // 2, (h % 2) * 64
                nc.vector.tensor_mul(
                    ctxT[mt][r0:r0 + 64, s0:s0 + Q],
                    ctx_ps[0:64, 0:Q], state["recbs"][i][:, 0:Q])

            return [p_recips, p_bcasts,
                    lambda: p_mul(0), lambda: p_mul(1)]

        def emit_norm_pair(ci, items):
            # final-pair norms: interleave the two heads' recip/broadcast/
            # multiply so the DVE and Pool stages pipeline instead of
            # serializing head-by-head at the kernel tail
            s0, Q, _ = CHUNKS[ci]
            recs = []
            for h, ctx_ps in items:
                rec = scr.tile([1, QC], FP32, tag="rec", name=f"r_{ci}_{h}")
                nc.vector.reciprocal(rec[:, 0:Q], ctx_ps[64:65, 0:Q])
                recs.append(rec)
            recbs = []
            for (h, _), rec in zip(items, recs):
                recb = scr.tile([64, QC], FP32, tag="recb",
                                name=f"rb_{ci}_{h}")
                nc.gpsimd.partition_broadcast(recb[:, 0:Q], rec[:, 0:Q])
                recbs.append(recb)
            for (h, ctx_ps), recb in zip(items, recbs):
                mt, r0 = h // 2, (h % 2) * 64
                nc.vector.tensor_mul(
                    ctxT[mt][r0:r0 + 64, s0:s0 + Q],
                    ctx_ps[0:64, 0:Q], recb[:, 0:Q])

        # ---- main schedule ----
        # exp-table warm: overlap the ~2.7us table load with the initial DMA
        warm = scr.tile([1, 1], FP32, tag="warm", bufs=1, name="warm")
        nc.gpsimd.memset(warm[:], 0.0)
        nc.scalar.activation(warm[:], warm[:], Exp)

        emit_proj_qk_interleaved(0, 0)
        pending_norms = []
        for ci in range(NCH):
            q0, Q, grp = CHUNKS[ci]
            nkb = (q0 + Q) // 128
            ngrp = nkb // grp
            waves = [(pair, g) for pair in range(2)
                     for g in range(ngrp)]
            head = []   # pieces pinned to the earliest waves, one per wave
            extra = []  # pieces distributed evenly over all waves
            pins = {}   # wave -> pieces with exact placement constraints
            # Each chunk's waves carry: its own V projections (head), the
            # NEXT chunk's full q/k projection + fused rearrange (extra, so
            # the DMA-staging chain completes well before that chunk's first
            # scores), and out-projection backlog.
            if ci == 0:
                qk0 = proj_qk_pieces(0)
                v0 = proj_v_pieces(range(0, 4))
                pins = {0: [qk0[2], v0[0]], 1: [qk0[3], v0[1]],
                        2: [v0[2], lambda: emit_rearrange(0, k_only=True)],
                        3: [v0[3]]}
                extra += proj_qk_pieces(1) + [lambda: emit_rearrange(1)]
            elif ci == 1:
                head += proj_v_pieces(range(4, 8))
                extra += proj_qk_pieces(2) + [lambda: emit_rearrange(2)]
            elif ci == 2:
                head += proj_v_pieces(range(8, 12))
                extra += (proj_qk_pieces(3) + [lambda: emit_rearrange(3)]
                          + outproj_pieces(0))
            elif ci == 3:
                head += proj_v_pieces(range(12, 16))
                extra += (proj_qk_pieces(4) + [lambda: emit_rearrange(4)]
                          + outproj_pieces(1))
            else:
                extra += outproj_pieces(2) + outproj_pieces(3)
            sched = {w: [] for w in range(len(waves))}
            for w, pcs in pins.items():
                sched[w].extend(pcs)
            for j, pc in enumerate(head):
                sched[j].append(pc)
            if extra:
                w0 = max(0, min(2 if ci == 0 else 4,
                                len(waves) - len(extra)))
                span_w = len(waves) - w0
                for j, pc in enumerate(extra):
                    sched[w0 + j * span_w // len(extra)].append(pc)

            ctx_tiles = {}
            ctx_queue = []
            for w, (pair, g) in enumerate(waves):
                if g == 0:
                    ctx_tiles[pair] = [
                        ps.tile([65, QC], FP32, tag="ctx", bufs=2,
                                name=f"c_{ci}_{pair}_{hh}")
                        for hh in range(2)]
                ex = wave_scores(ci, pair, g)
                if pending_norms:
                    pending_norms.pop(0)()
                last_of_pair = g == ngrp - 1
                final_pair = last_of_pair and pair == 1 and ci + 1 == NCH
                if not final_pair:
                    for pc in sched[w]:
                        pc()
                ctx_queue.append((pair, g, ex))
                # defer ctx so (a) the pair's first ctx matmuls don't stall
                # on PSUM slots still being normed, (b) PE has scores to run
                # while exp catches up. Grouped chunks defer until wave 3 so
                # the previous pair's lazily-spread norm muls (waves 2,3)
                # have released the slots.
                if grp > 1:
                    lag = max(0, 3 - g)
                else:
                    lag = 4 if g < 4 else 0
                while len(ctx_queue) > lag or \
                        (ctx_queue and last_of_pair):
                    qpair, qg, qex = ctx_queue.pop(0)
                    wave_ctx(ci, qpair, qg, qex, ctx_tiles[qpair], nkb)
                if last_of_pair:
                    h0 = 2 * pair
                    items = [(h0 + hh, ctx_tiles[pair][hh])
                             for hh in range(2)]
                    if final_pair:
                        # kernel tail: emit the norm chain ahead of this
                        # wave's filler copies so the recips don't queue
                        # behind them on the in-order DVE
                        emit_norm_pair(ci, items)
                        for pc in sched[w]:
                            pc()
                    else:
                        # lazily spread over the next 4 waves (pair 0's run
                        # inside this chunk's pair-1 waves; pair 1's inside
                        # the next chunk)
                        pending_norms = norm_pieces(ci, items)
        for pc in outproj_pieces(NCH - 1, tail=True):
            pc()


def build_module():
    nc = bacc.Bacc("TRN2", target_bir_lowering=False, debug=False)
    xT = nc.dram_tensor("xT", [E, S], BF16, kind="ExternalInput").ap()
    wq = nc.dram_tensor("wq", [E, M], BF16, kind="ExternalInput").ap()
    wk = nc.dram_tensor("wk", [E, M], BF16, kind="ExternalInput").ap()
    wv = nc.dram_tensor("wv", [E, M], BF16, kind="ExternalInput").ap()
    wo = nc.dram_tensor("wo", [M, E], BF16, kind="ExternalInput").ap()
    x8 = nc.dram_tensor("x8", [128, 8 * S], F8, kind="ExternalInput").ap()
    wk8 = nc.dram_tensor("wk8", [128, 8 * M], F8, kind="ExternalInput").ap()
    out = nc.dram_tensor("out", [S, E], BF16, kind="ExternalOutput").ap()
    with tile.TileContext(nc) as tc:
        _emit_kernel(tc, xT, wq, wk, wv, wo, x8, wk8, out)
    nc.compile()
    return nc


def _pack_epairs(aT):
    """[E, N] -> [128, 4*2*N] fp8: e-tile pairs side by side per partition
    (DoubleRow packing: out[p, j, t, n] = aT[(2j+t)*128 + p, n])."""
    e4m3 = ml_dtypes.float8_e4m3
    E_, N = aT.shape
    a = np.asarray(aT, dtype=np.float32).reshape(4, 2, 128, N)
    a = np.ascontiguousarray(a.transpose(2, 0, 1, 3)).astype(e4m3)
    return a.reshape(128, 8 * N)


def make_in_maps(x, w_qkv):
    """Per-core input dicts (bf16/fp8, pre-transposed host-side)."""
    bf = ml_dtypes.bfloat16
    xTb = [np.ascontiguousarray(x[b].T).astype(bf) for b in range(B)]
    x8b = [_pack_epairs(x[b].T) for b in range(B)]
    in_maps = []
    for c in range(NCORES):
        b, g = c // 4, c % 4
        cols = slice(g * M, (g + 1) * M)
        wkT = np.ascontiguousarray(w_qkv[E:][cols, :].T)
        in_maps.append({
            "xT": xTb[b],
            "wq": np.ascontiguousarray(w_qkv[cols, :].T).astype(bf),
            "wk": wkT.astype(bf),
            "wv": np.ascontiguousarray(w_qkv[2 * E:][cols, :].T).astype(bf),
            "x8": x8b[b],
            "wk8": _pack_epairs(wkT * 64.0),
            "wo": None,  # filled in kernel(), needs w_out
        })
    return in_maps


_RUNNER = None
_SHARDED = None


def _get_runner():
    """Build the Bass module once and return a cached callable
    (in_maps) -> [NCORES, S, E] bf16 partial outputs."""
    global _RUNNER
    if _RUNNER is not None:
        return _RUNNER

    nc = build_module()

    from concourse import bass2jax
    import jax
    from jax.sharding import Mesh, PartitionSpec
    from jax.experimental.shard_map import shard_map

    bass2jax.install_neuronx_cc_hook()

    in_names = ["xT", "wq", "wk", "wv", "x8", "wk8", "wo"]
    out_names = ["out"]
    out_avals = [jax.core.ShapedArray((S, E), ml_dtypes.bfloat16)]
    n_params = len(in_names)
    all_names = in_names + out_names
    partition_name = (nc.partition_id_tensor.name
                      if nc.partition_id_tensor is not None else None)
    if partition_name is not None:
        all_names = all_names + [partition_name]

    def _body(*args):
        operands = list(args)
        if partition_name is not None:
            operands.append(bass2jax.partition_id_tensor())
        outs = bass2jax._bass_exec_p.bind(
            *operands,
            out_avals=tuple(out_avals),
            in_names=tuple(all_names),
            out_names=tuple(out_names),
            lowering_input_output_aliases=(),
            sim_require_finite=True,
            sim_require_nnan=True,
            nc=nc,
        )
        return tuple(outs)

    devices = jax.devices()[:NCORES]
    mesh = Mesh(np.asarray(devices), ("core",))
    n_outs = len(out_names)
    in_specs = (PartitionSpec("core"),) * (n_params + n_outs)
    out_specs = (PartitionSpec("core"),) * n_outs
    sharded = jax.jit(
        shard_map(_body, mesh=mesh, in_specs=in_specs, out_specs=out_specs,
                  check_rep=False),
        donate_argnums=tuple(range(n_params, n_params + n_outs)),
        keep_unused=True,
    )
    global _SHARDED
    _SHARDED = sharded

    def run(in_maps):
        concat_in = [
            np.concatenate([np.asarray(in_maps[c][n]) for c in range(NCORES)],
                           axis=0)
            for n in in_names
        ]
        concat_zeros = [np.zeros((NCORES * S, E), ml_dtypes.bfloat16)]
        out_arrs = sharded(*concat_in, *concat_zeros)
        return np.asarray(out_arrs[0]).reshape(NCORES, S, E)

    _RUNNER = run
    return run


def kernel(x, w_qkv, w_out, b_out):
    x = np.asarray(x, dtype=np.float32)
    w_qkv = np.asarray(w_qkv, dtype=np.float32)
    w_out = np.asarray(w_out, dtype=np.float32)
    b_out = np.asarray(b_out, dtype=np.float32)

    bf = ml_dtypes.bfloat16
    in_maps = make_in_maps(x, w_qkv)
    for c in range(NCORES):
        g = c % 4
        cols = slice(g * M, (g + 1) * M)
        in_maps[c]["wo"] = np.ascontiguousarray(w_out[:, cols].T).astype(bf)

    run = _get_runner()
    partials = run(in_maps)  # [8, S, E] bf16

    out = np.empty((B, S, E), np.float32)
    for b in range(B):
        acc = partials[4 * b].astype(np.float64)
        for i in range(1, 4):
            acc += partials[4 * b + i].astype(np.float64)
        out[b] = (acc + b_out.astype(np.float64)).astype(np.float32)
    return out
